# revision 1
# baseline (speedup 1.0000x reference)
"""BitLinear (quantized-activation, binarized-sprinkled-weight linear) Trainium2 kernel.

Data-parallel over the token dim N across 8 NeuronCores. Each core:
  * quantize-blends its x shard with one fused custom DVE op  -> xb bf16
  * sign/sprinkle/blends the full weight with one fused custom DVE op -> W2 bf16
    (final_scale, post_bin_scale and the activation blend scale folded into
     per-partition scalars; weight DMA-cast to bf16 on load; mask read as u8)
  * puts the contraction dim on partitions: W2 via batched xbar DMA-transposes
    (one [128,2048] op per o-block), xb via TensorE transpose + ScalarE copy
    (cheaper than DMA here, and it keeps the PE clock warm before the matmuls)
  * 512 bf16 matmuls (K=128, N=512 via 3D strided rhs APs) accumulating in PSUM
  * DVE adds the broadcast final_bias while copying PSUM->SBUF, DMA out.
Streams are spread over the DMA queues (w cast-loads on gpsimd/SWDGE, masks and
output stores on the scalar HWDGE ring, x loads and w transposes on sync).

Math: reference out = xq @ w_final^T * fs + fb with
  xq      = 0.5*x + 0.5*s*clip(round(x/(s+eps)), +-127)        (s = running_max/127)
  w_final = m ? h : 0.5*(w + h),  h = sign(w)*pbs
Here  xb = x*inv_se + clip(round(x*inv_se), +-127)  with inv_se = 1/(s+eps), so
  xq ~= sigma*xb with sigma = 0.5*(s+eps)   (error <= 0.5*eps*127 ~ 6e-5 absolute)
and the device computes  out = xb @ W2^T + fb  with W2 = sigma*fs*w_final:
  W2 = m ? sign(w)*C0 : w*C1 + sign(w)*C0*0.5,
  C0[o] = sigma*fs[o]*pbs[o],  C1[o] = 0.5*sigma*fs[o].
"""

import numpy as np

N_CORES = 8
N_TOK, D_IN, D_OUT = 8192, 2048, 2048
N_SHARD = N_TOK // N_CORES          # 1024 rows of x per core
P = 128
NJ = N_SHARD // P                   # 8 n-blocks per core
NB_I = D_IN // P                    # 16 i-blocks (contraction)
NB_O = D_OUT // P                   # 16 o-blocks
OT = 512                            # o-tile (one PSUM bank)
NT = D_OUT // OT                    # 4 o-tiles
OB_PER_T = OT // P                  # 4 o-blocks per o-tile

QMAX = 127.0
EPS = 1e-6
MAGIC = 12582912.0                  # 1.5 * 2**23: fp32 RNE round-to-int trick

_CACHE = {}


def _register_ops():
    """Define the two fused DVE ops (idempotent)."""
    from concourse import dve_ops
    from concourse.dve_spec import (
        Spec, Src0, Src1, C0, C1, C2, Zero, select, minn, maxx, lower, _has_src1,
    )
    from concourse.dve_uop import DveOpSpec

    def register(name, spec):
        for op in dve_ops.OPS:
            if op.name == name:
                return op
        ver = "v3"
        tmp = DveOpSpec(name=name, opcode=0, uops=lower(spec, ver=ver),
                        rd1_en=_has_src1(spec))
        op = dve_ops.DveOp(name, spec, subdim=False,
                           uops_sha={ver: tmp.sha(ver)})
        dve_ops.OPS.append(op)
        dve_ops._SUB_OPCODE_FOR_NAME[name] = (
            max(dve_ops._SUB_OPCODE_FOR_NAME.values()) + 1)
        dve_ops.CUSTOM_DVE_SPECS[name] = spec
        return op

    # out = t + clip(round(t), +-imm2), t = x*s0   (s1 = MAGIC)
    _t = Src0 * C0
    _r = (_t + C1) - C1
    _rc = minn(maxx(_r, Zero - C2), C2)
    xprep = register("XPREP_BITLIN", Spec(
        body=_t + _rc,
        reference=lambda in0, in1, s0, s1, imm2: (
            (lambda t: t + np.clip(np.round(t), -imm2, imm2))(
                in0.astype(np.float32) * s0)),
    ))

    # h = select(w>=0, s0, -s0); out = select(m>0, h, w*s1 + h*imm2)
    _h = select(Src0 >= Zero, C0, Zero - C0)
    wprep = register("WPREP_BITLIN", Spec(
        body=select(Src1 > Zero, _h, Src0 * C1 + _h * C2),
        reference=lambda in0, in1, s0, s1, imm2: (
            (lambda h: np.where(in1 > 0, h,
                                in0.astype(np.float32) * s1 + h * imm2))(
                np.where(in0 >= 0, s0, -s0))),
    ))
    return xprep, wprep


def _build(inv_se):
    """Build + compile the per-core Bass module. inv_se is baked in."""
    key = ("nc", float(inv_se))
    if key in _CACHE:
        return _CACHE[key]

    import concourse.mybir as mybir
    import concourse.tile as tile
    from concourse import bacc

    xprep, wprep = _register_ops()

    nc = bacc.Bacc(None, target_bir_lowering=False)
    bf16 = mybir.dt.bfloat16
    f32 = mybir.dt.float32

    x_in = nc.dram_tensor("x", [N_SHARD, D_IN], f32, kind="ExternalInput")
    w_in = nc.dram_tensor("w", [D_OUT, D_IN], f32, kind="ExternalInput")
    m_in = nc.dram_tensor("m", [D_OUT, D_IN], mybir.dt.uint8, kind="ExternalInput")
    c0_in = nc.dram_tensor("c0", [P, NB_O], f32, kind="ExternalInput")
    c1_in = nc.dram_tensor("c1", [P, NB_O], f32, kind="ExternalInput")
    fb_in = nc.dram_tensor("fb", [P, D_OUT], f32, kind="ExternalInput")
    out_o = nc.dram_tensor("out", [N_SHARD, D_OUT], f32, kind="ExternalOutput")

    from concourse.masks import make_identity

    with tile.TileContext(nc) as tc:
        with (
            tc.tile_pool(name="persist", bufs=1) as persist,
            tc.tile_pool(name="wlp", bufs=4) as wlp,
            tc.tile_pool(name="wpp", bufs=4) as wpp,
            tc.tile_pool(name="xlp", bufs=4) as xlp,
            tc.tile_pool(name="xbp", bufs=4) as xbp,
            tc.tile_pool(name="ostage", bufs=7) as ostage,
            tc.tile_pool(name="psum", bufs=6, space="PSUM") as psum,
            tc.tile_pool(name="tpsum", bufs=2, space="PSUM") as tpsum,
        ):
            # persistent operand tiles
            wT = persist.tile([P, NB_O, NB_I, P], bf16, tag="wT")     # [i_in, ob, ib, o_in]
            xqT = persist.tile([P, NJ, NB_I, P], bf16, tag="xqT")     # [i_in, j, ib, n_in]
            c0_sb = persist.tile([P, NB_O], f32, tag="c0")
            c1_sb = persist.tile([P, NB_O], f32, tag="c1")
            fb_sb = persist.tile([P, D_OUT], f32, tag="fb")
            ident = persist.tile([P, P], bf16, tag="ident")

            nc.sync.dma_start(fb_sb[:], fb_in[:])
            nc.sync.dma_start(c0_sb[:], c0_in[:])
            nc.sync.dma_start(c1_sb[:], c1_in[:])
            make_identity(nc, ident[:])

            def w_block(ob):
                wt = wlp.tile([P, D_IN], bf16, tag="w_bf16")
                mt = wlp.tile([P, D_IN], mybir.dt.uint8, tag="m_u8")
                nc.gpsimd.dma_start(wt[:], w_in[ob * P:(ob + 1) * P, :])   # f32->bf16
                nc.scalar.dma_start(mt[:], m_in[ob * P:(ob + 1) * P, :])
                w2 = wpp.tile([P, D_IN], bf16, tag="w2")
                nc.vector._custom_dve(
                    wprep, out=w2[:], in0=wt[:], in1=mt[:],
                    s0=c0_sb[:, ob:ob + 1], s1=c1_sb[:, ob:ob + 1], imm2=0.5)
                nc.sync.dma_start_transpose(wT[:, ob], w2[:])

            def x_block(j):
                xt = xlp.tile([P, D_IN], f32, tag="x_f32")
                nc.sync.dma_start(xt[:], x_in[j * P:(j + 1) * P, :])
                xb = xbp.tile([P, D_IN], bf16, tag="xb")
                nc.vector._custom_dve(
                    xprep, out=xb[:], in0=xt[:],
                    s0=float(inv_se), s1=MAGIC, imm2=QMAX)
                # transpose on the (otherwise idle-at-this-point) PE + ScalarE
                for b in range(NB_I):
                    tp = tpsum.tile([P, P], bf16, tag="xtp")
                    nc.tensor.transpose(tp[:], xb[:, b * P:(b + 1) * P], ident[:])
                    nc.scalar.copy(xqT[:, j, b, :], tp[:])

            # weight blocks for the first o-tile, then x, then the rest
            for ob in range(OB_PER_T):
                w_block(ob)
            for j in range(NJ):
                x_block(j)
            for ob in range(OB_PER_T, NB_O):
                w_block(ob)

            for t in range(NT):
                for j in range(NJ):
                    ps = psum.tile([P, OT], f32, tag="ps")
                    for b in range(NB_I):
                        nc.tensor.matmul(
                            ps[:],
                            xqT[:, j, b, :],
                            wT[:, t * OB_PER_T:(t + 1) * OB_PER_T, b, :],
                            start=(b == 0), stop=(b == NB_I - 1))
                    osb = ostage.tile([P, OT], f32, tag="osb")
                    nc.vector.tensor_add(
                        osb[:], ps[:], fb_sb[:, t * OT:(t + 1) * OT])
                    nc.scalar.dma_start(
                        out_o[j * P:(j + 1) * P, t * OT:(t + 1) * OT], osb[:])

    nc.compile()
    _CACHE[key] = nc
    return nc


def _in_maps(x, weight, mask_u8, c0, c1, fb):
    maps = []
    for c in range(N_CORES):
        maps.append({
            "x": np.ascontiguousarray(x[c * N_SHARD:(c + 1) * N_SHARD]),
            "w": weight,
            "m": mask_u8,
            "c0": c0,
            "c1": c1,
            "fb": fb,
        })
    return maps


def _host_consts(post_bin_scale, final_scale, final_bias, running_max):
    s = np.float32(running_max) / np.float32(QMAX)
    inv_se = np.float32(1.0) / (s + np.float32(EPS))
    sigma = np.float64(0.5) * (np.float64(s) + np.float64(EPS))
    c0_all = (sigma * final_scale.astype(np.float64)
              * post_bin_scale.reshape(-1).astype(np.float64)).astype(np.float32)
    c1_all = (np.float64(0.5) * sigma
              * final_scale.astype(np.float64)).astype(np.float32)
    # [o] -> [p, ob] with o = ob*128 + p
    c0 = np.ascontiguousarray(c0_all.reshape(NB_O, P).T)
    c1 = np.ascontiguousarray(c1_all.reshape(NB_O, P).T)
    fb = np.ascontiguousarray(
        np.broadcast_to(final_bias.astype(np.float32), (P, D_OUT)))
    return inv_se, c0, c1, fb


def kernel(x, weight, post_bin_scale, final_scale, final_bias, running_max,
           sprinkle_mask):
    from concourse.bass_utils import run_bass_kernel_spmd

    x = np.asarray(x, dtype=np.float32)
    weight = np.ascontiguousarray(np.asarray(weight, dtype=np.float32))
    mask_u8 = np.ascontiguousarray(np.asarray(sprinkle_mask)).view(np.uint8)
    inv_se, c0, c1, fb = _host_consts(
        np.asarray(post_bin_scale, dtype=np.float32),
        np.asarray(final_scale, dtype=np.float32),
        np.asarray(final_bias, dtype=np.float32),
        float(np.asarray(running_max)))

    nc = _build(inv_se)
    maps = _in_maps(x, weight, mask_u8, c0, c1, fb)

    # The axon-tunneled devices can transiently fail
    # (NRT_EXEC_UNIT_UNRECOVERABLE); a fresh PJRT client recovers. Retry the
    # execute with a backend reset rather than failing the whole call.
    last_exc = None
    for attempt in range(3):
        try:
            res = run_bass_kernel_spmd(nc, maps, core_ids=list(range(N_CORES)))
            break
        except Exception as exc:  # noqa: BLE001 - retrying device-side faults
            last_exc = exc
            if attempt == 2:
                raise
            import gc
            import time as _time
            gc.collect()
            try:
                import jax
                jax.clear_caches()
                import jax.extend as _jex
                _jex.backend.clear_backends()
            except Exception:
                pass
            _time.sleep(10)
    out = np.concatenate([res.results[c]["out"] for c in range(N_CORES)], axis=0)
    return out



# revision 10
# speedup vs baseline: 1.1941x; 1.1941x over previous
"""BitLinear (quantized-activation, binarized-sprinkled-weight linear) Trainium2 kernel.

Data-parallel over the token dim N across 8 NeuronCores.

Fast path (post_bin_scale uniform, which holds for this problem's inputs):
  * all operands are laid out HOST-side so the contraction dim i lands on
    SBUF partitions with no device transposes at all:
       xT [IN, N_SHARD] f32 (per core),  wT [IN, OUT] f32,  mT [IN, OUT] u16
  * w/x are DMA-cast to bf16 on load (gpsimd SWDGE); mask is u16 so every
    DVE operand is 2-byte -> double-pumped custom DVE ops
  * one fused DVE op per operand:
       xb = t + clip(round(t), +-127),  t = x*inv_se          (bf16)
       W2 = m ? sign(w)*C0 : w*C1 + sign(w)*C0*0.5            (bf16)
    with C0 = sigma*pbs, C1 = 0.5*sigma global scalars (sigma = 0.5*(s+eps)),
    so that  out[n,o] = (xb @ W2^T)*fs[o] + fb[o]
  * matmuls produce PSUM tiles [o=128, n=512] (o on partitions), 16 k-steps
    each, issued in waves of 8 interleaved k-outer so the PE can consume
    operand blocks as the DVE produces them
  * epilogue on the Activation engine: Identity with per-partition scale=fs,
    bias=fb, writing bf16; stores go out transposed [OUT, N_SHARD] and the
    host transposes back (layout-only work).

General path (non-uniform post_bin_scale): previous per-partition-constant
implementation, kept verbatim as a fallback.
"""

import numpy as np

N_CORES = 8
N_TOK, D_IN, D_OUT = 8192, 2048, 2048
N_SHARD = N_TOK // N_CORES          # 1024 rows of x per core
P = 128
NJ = N_SHARD // P                   # 8 n-blocks per core
NB_I = D_IN // P                    # 16 i-blocks (contraction)
NB_O = D_OUT // P                   # 16 o-blocks
OT = 512                            # o-tile (one PSUM bank)
NT = D_OUT // OT                    # 4 o-tiles
OB_PER_T = OT // P                  # 4 o-blocks per o-tile

MM_NT = N_SHARD // 512              # fast path: 2 moving-dim tiles of 512

QMAX = 127.0
EPS = 1e-6
MAGIC = 12582912.0                  # 1.5 * 2**23: fp32 RNE round-to-int trick

_CACHE = {}


def _register_ops():
    """Define the two fused DVE ops (idempotent)."""
    from concourse import dve_ops
    from concourse.dve_spec import (
        Spec, Src0, Src1, C0, C1, C2, Zero, select, minn, maxx, lower, _has_src1,
    )
    from concourse.dve_uop import DveOpSpec

    def register(name, spec):
        for op in dve_ops.OPS:
            if op.name == name:
                return op
        ver = "v3"
        tmp = DveOpSpec(name=name, opcode=0, uops=lower(spec, ver=ver),
                        rd1_en=_has_src1(spec))
        op = dve_ops.DveOp(name, spec, subdim=False,
                           uops_sha={ver: tmp.sha(ver)})
        dve_ops.OPS.append(op)
        dve_ops._SUB_OPCODE_FOR_NAME[name] = (
            max(dve_ops._SUB_OPCODE_FOR_NAME.values()) + 1)
        dve_ops.CUSTOM_DVE_SPECS[name] = spec
        return op

    # out = t + clip(round(t), +-imm2), t = x*s0   (s1 = MAGIC)
    _t = Src0 * C0
    _r = (_t + C1) - C1
    _rc = minn(maxx(_r, Zero - C2), C2)
    xprep = register("XPREP_BITLIN", Spec(
        body=_t + _rc,
        reference=lambda in0, in1, s0, s1, imm2: (
            (lambda t: t + np.clip(np.round(t), -imm2, imm2))(
                in0.astype(np.float32) * s0)),
    ))

    # h = select(w>=0, s0, -s0); out = select(m>0, h, w*s1 + h*imm2)
    _h = select(Src0 >= Zero, C0, Zero - C0)
    wprep = register("WPREP_BITLIN", Spec(
        body=select(Src1 > Zero, _h, Src0 * C1 + _h * C2),
        reference=lambda in0, in1, s0, s1, imm2: (
            (lambda h: np.where(in1 > 0, h,
                                in0.astype(np.float32) * s1 + h * imm2))(
                np.where(in0 >= 0, s0, -s0))),
    ))
    return xprep, wprep


def _build_fast(inv_se, c0s, c1s):
    """Fast-path per-core Bass module (uniform post_bin_scale).

    inv_se, c0s, c1s are global scalars baked into the DVE ops.
    """
    key = ("fast", float(inv_se), float(c0s), float(c1s))
    if key in _CACHE:
        return _CACHE[key]

    import concourse.mybir as mybir
    import concourse.tile as tile
    from concourse import bacc

    xprep, wprep = _register_ops()

    nc = bacc.Bacc(None, target_bir_lowering=False)
    bf16 = mybir.dt.bfloat16
    f32 = mybir.dt.float32
    ident = mybir.ActivationFunctionType.Identity

    x_in = nc.dram_tensor("x", [D_IN, N_SHARD], f32, kind="ExternalInput")
    w_in = nc.dram_tensor("w", [D_IN, D_OUT], f32, kind="ExternalInput")
    m_in = nc.dram_tensor("m", [D_IN, D_OUT], mybir.dt.uint8, kind="ExternalInput")
    fs_in = nc.dram_tensor("fs", [P, NB_O], f32, kind="ExternalInput")
    fb_in = nc.dram_tensor("fb", [P, NB_O], f32, kind="ExternalInput")
    out_o = nc.dram_tensor("out", [D_OUT, N_SHARD], bf16, kind="ExternalOutput")

    KH = NB_I // 2                  # contraction split: 8 + 8 k-steps
    tiles = [(ob, nt) for nt in range(MM_NT) for ob in range(NB_O)]
    waves = [tiles[w0:w0 + 8] for w0 in range(0, len(tiles), 8)]

    with tile.TileContext(nc) as tc:
        with (
            tc.tile_pool(name="persist", bufs=1) as persist,
            tc.tile_pool(name="wlp", bufs=4) as wlp,
            tc.tile_pool(name="xlp", bufs=4) as xlp,
            tc.tile_pool(name="ostage", bufs=8) as ostage,
            tc.tile_pool(name="psum", bufs=8, space="PSUM") as psum,
        ):
            w2 = persist.tile([P, NB_I, D_OUT], bf16, tag="w2")   # [i_in, ib, o]
            xb = persist.tile([P, NB_I, N_SHARD], bf16, tag="xb")  # [i_in, ib, n]
            # bf16 partial sums for the first contraction half (PSUM is only
            # 8 banks; parking partials in SBUF lets all 32 output tiles
            # overlap the operand-prep phase)
            parts = persist.tile([P, len(tiles), 512], bf16, tag="parts")
            fs_sb = persist.tile([P, NB_O], f32, tag="fs")
            fb_sb = persist.tile([P, NB_O], f32, tag="fb")

            nc.sync.dma_start(fs_sb[:], fs_in[:])
            nc.sync.dma_start(fb_sb[:], fb_in[:])

            # operand prep, interleaved so matmul waves can start early
            for ib in range(NB_I):
                xt = xlp.tile([P, N_SHARD], bf16, tag="x_bf16")
                nc.gpsimd.dma_start(xt[:], x_in[ib * P:(ib + 1) * P, :])  # cast
                mt = wlp.tile([P, D_OUT], mybir.dt.uint8, tag="m_u8")
                nc.scalar.dma_start(mt[:], m_in[ib * P:(ib + 1) * P, :])
                wt = wlp.tile([P, D_OUT], bf16, tag="w_bf16")
                nc.gpsimd.dma_start(wt[:], w_in[ib * P:(ib + 1) * P, :])  # cast
                nc.vector._custom_dve(
                    xprep, out=xb[:, ib, :], in0=xt[:],
                    s0=float(inv_se), s1=MAGIC, imm2=QMAX)
                nc.vector._custom_dve(
                    wprep, out=w2[:, ib, :], in0=wt[:], in1=mt[:],
                    s0=float(c0s), s1=float(c1s), imm2=0.5)

            def mm(ps, ob, nt, ib, start, stop):
                nc.tensor.matmul(
                    ps[:],
                    w2[:, ib, ob * P:(ob + 1) * P],
                    xb[:, ib, nt * 512:(nt + 1) * 512],
                    start=start, stop=stop, skip_group_check=True)

            # phase 1 (overlaps prep): first contraction half for every tile,
            # k-outer within each 8-tile wave; partials parked in SBUF bf16.
            # The last wave's final 4 tiles keep their banks: phase 2
            # continues them in place while the first reloads pipeline in.
            held = []
            for wi, wave in enumerate(waves):
                pss = [psum.tile([P, 512], f32, name="ps", tag="ps")
                       for _ in wave]
                for ib in range(KH):
                    for ps, (ob, nt) in zip(pss, wave):
                        mm(ps, ob, nt, ib, ib == 0, ib == KH - 1)
                if wi == len(waves) - 1:
                    for k in range(4):
                        nc.scalar.copy(parts[:, wi * 8 + k, :], pss[k][:])
                    held = pss[4:]
                else:
                    for k, ps in enumerate(pss):
                        nc.scalar.copy(parts[:, wi * 8 + k, :], ps[:])

            def epilogue(ps, ob, nt):
                osb = ostage.tile([P, 512], bf16, tag="osb")
                nc.scalar.activation(
                    osb[:], ps[:], ident,
                    bias=fb_sb[:, ob:ob + 1], scale=fs_sb[:, ob:ob + 1])
                nc.sync.dma_start(
                    out_o[ob * P:(ob + 1) * P, nt * 512:(nt + 1) * 512],
                    osb[:])

            # phase 2: finish the 4 held tiles first (banks already hot) while
            # the first reload wave pipelines in on the Act queue; then parked
            # tiles in small waves, reloads running ahead of epilogues.
            parked = tiles[:28]
            held_tiles = tiles[28:]
            sizes = [4, 4, 4, 4, 4, 4, 2, 2]
            p2, pos = [], 0
            for sz in sizes:
                p2.append(list(enumerate(parked[pos:pos + sz], start=pos)))
                pos += sz

            def load_wave(wi):
                pss = [psum.tile([P, 512], f32, name="ps", tag="ps")
                       for _ in p2[wi]]
                for ps, (idx, _) in zip(pss, p2[wi]):
                    nc.scalar.copy(ps[:], parts[:, idx, :])
                return pss

            live = {0: load_wave(0)}
            for ib in range(KH, NB_I):
                for ps, (ob, nt) in zip(held, held_tiles):
                    mm(ps, ob, nt, ib, False, ib == NB_I - 1)
            for ps, (ob, nt) in zip(held, held_tiles):
                epilogue(ps, ob, nt)
            live[1] = load_wave(1)

            for wi, wave in enumerate(p2):
                pss = live.pop(wi)
                for ib in range(KH, NB_I):
                    for ps, (_, (ob, nt)) in zip(pss, wave):
                        mm(ps, ob, nt, ib, False, ib == NB_I - 1)
                for ps, (_, (ob, nt)) in zip(pss, wave):
                    epilogue(ps, ob, nt)
                if wi + 2 < len(p2):
                    live[wi + 2] = load_wave(wi + 2)

    nc.compile()
    _CACHE[key] = nc
    return nc


def _fast_consts(post_bin_scale, final_scale, final_bias, running_max):
    s = np.float32(running_max) / np.float32(QMAX)
    inv_se = np.float32(1.0) / (s + np.float32(EPS))
    sigma = np.float64(0.5) * (np.float64(s) + np.float64(EPS))
    pbs0 = np.float64(post_bin_scale.reshape(-1)[0])
    c0s = np.float32(sigma * pbs0)
    c1s = np.float32(np.float64(0.5) * sigma)
    fscol = np.ascontiguousarray(
        final_scale.astype(np.float32).reshape(NB_O, P).T)
    fbcol = np.ascontiguousarray(
        final_bias.astype(np.float32).reshape(NB_O, P).T)
    return inv_se, c0s, c1s, fscol, fbcol


def _fast_in_maps(x, weight, mask, fscol, fbcol):
    wT = np.ascontiguousarray(weight.T)
    mT = np.ascontiguousarray(mask.T).view(np.uint8)
    maps = []
    for c in range(N_CORES):
        maps.append({
            "x": np.ascontiguousarray(x[c * N_SHARD:(c + 1) * N_SHARD].T),
            "w": wT,
            "m": mT,
            "fs": fscol,
            "fb": fbcol,
        })
    return maps


# ---------------------------------------------------------------------------
# general fallback (non-uniform post_bin_scale): previous implementation
# ---------------------------------------------------------------------------

def _build_general(inv_se):
    key = ("nc", float(inv_se))
    if key in _CACHE:
        return _CACHE[key]

    import concourse.mybir as mybir
    import concourse.tile as tile
    from concourse import bacc

    xprep, wprep = _register_ops()

    nc = bacc.Bacc(None, target_bir_lowering=False)
    bf16 = mybir.dt.bfloat16
    f32 = mybir.dt.float32

    x_in = nc.dram_tensor("x", [N_SHARD, D_IN], f32, kind="ExternalInput")
    w_in = nc.dram_tensor("w", [D_OUT, D_IN], f32, kind="ExternalInput")
    m_in = nc.dram_tensor("m", [D_OUT, D_IN], mybir.dt.uint8, kind="ExternalInput")
    c0_in = nc.dram_tensor("c0", [P, NB_O], f32, kind="ExternalInput")
    c1_in = nc.dram_tensor("c1", [P, NB_O], f32, kind="ExternalInput")
    fb_in = nc.dram_tensor("fb", [P, D_OUT], f32, kind="ExternalInput")
    out_o = nc.dram_tensor("out", [N_SHARD, D_OUT], f32, kind="ExternalOutput")

    from concourse.masks import make_identity

    with tile.TileContext(nc) as tc:
        with (
            tc.tile_pool(name="persist", bufs=1) as persist,
            tc.tile_pool(name="wlp", bufs=4) as wlp,
            tc.tile_pool(name="wpp", bufs=4) as wpp,
            tc.tile_pool(name="xlp", bufs=4) as xlp,
            tc.tile_pool(name="xbp", bufs=4) as xbp,
            tc.tile_pool(name="ostage", bufs=7) as ostage,
            tc.tile_pool(name="psum", bufs=6, space="PSUM") as psum,
            tc.tile_pool(name="tpsum", bufs=2, space="PSUM") as tpsum,
        ):
            wT = persist.tile([P, NB_O, NB_I, P], bf16, tag="wT")
            xqT = persist.tile([P, NJ, NB_I, P], bf16, tag="xqT")
            c0_sb = persist.tile([P, NB_O], f32, tag="c0")
            c1_sb = persist.tile([P, NB_O], f32, tag="c1")
            fb_sb = persist.tile([P, D_OUT], f32, tag="fb")
            ident = persist.tile([P, P], bf16, tag="ident")

            nc.sync.dma_start(fb_sb[:], fb_in[:])
            nc.sync.dma_start(c0_sb[:], c0_in[:])
            nc.sync.dma_start(c1_sb[:], c1_in[:])
            make_identity(nc, ident[:])

            def w_block(ob):
                wt = wlp.tile([P, D_IN], bf16, tag="w_bf16")
                mt = wlp.tile([P, D_IN], mybir.dt.uint8, tag="m_u8")
                nc.gpsimd.dma_start(wt[:], w_in[ob * P:(ob + 1) * P, :])
                nc.scalar.dma_start(mt[:], m_in[ob * P:(ob + 1) * P, :])
                w2 = wpp.tile([P, D_IN], bf16, tag="w2")
                nc.vector._custom_dve(
                    wprep, out=w2[:], in0=wt[:], in1=mt[:],
                    s0=c0_sb[:, ob:ob + 1], s1=c1_sb[:, ob:ob + 1], imm2=0.5)
                nc.sync.dma_start_transpose(wT[:, ob], w2[:])

            def x_block(j):
                xt = xlp.tile([P, D_IN], f32, tag="x_f32")
                nc.sync.dma_start(xt[:], x_in[j * P:(j + 1) * P, :])
                xb = xbp.tile([P, D_IN], bf16, tag="xb")
                nc.vector._custom_dve(
                    xprep, out=xb[:], in0=xt[:],
                    s0=float(inv_se), s1=MAGIC, imm2=QMAX)
                for b in range(NB_I):
                    tp = tpsum.tile([P, P], bf16, tag="xtp")
                    nc.tensor.transpose(tp[:], xb[:, b * P:(b + 1) * P], ident[:])
                    nc.scalar.copy(xqT[:, j, b, :], tp[:])

            for ob in range(OB_PER_T):
                w_block(ob)
            for j in range(NJ):
                x_block(j)
            for ob in range(OB_PER_T, NB_O):
                w_block(ob)

            for t in range(NT):
                for j in range(NJ):
                    ps = psum.tile([P, OT], f32, tag="ps")
                    for b in range(NB_I):
                        nc.tensor.matmul(
                            ps[:],
                            xqT[:, j, b, :],
                            wT[:, t * OB_PER_T:(t + 1) * OB_PER_T, b, :],
                            start=(b == 0), stop=(b == NB_I - 1))
                    osb = ostage.tile([P, OT], f32, tag="osb")
                    nc.vector.tensor_add(
                        osb[:], ps[:], fb_sb[:, t * OT:(t + 1) * OT])
                    nc.scalar.dma_start(
                        out_o[j * P:(j + 1) * P, t * OT:(t + 1) * OT], osb[:])

    nc.compile()
    _CACHE[key] = nc
    return nc


def _general_in_maps(x, weight, mask_u8, c0, c1, fb):
    maps = []
    for c in range(N_CORES):
        maps.append({
            "x": np.ascontiguousarray(x[c * N_SHARD:(c + 1) * N_SHARD]),
            "w": weight,
            "m": mask_u8,
            "c0": c0,
            "c1": c1,
            "fb": fb,
        })
    return maps


def _general_consts(post_bin_scale, final_scale, final_bias, running_max):
    s = np.float32(running_max) / np.float32(QMAX)
    inv_se = np.float32(1.0) / (s + np.float32(EPS))
    sigma = np.float64(0.5) * (np.float64(s) + np.float64(EPS))
    c0_all = (sigma * final_scale.astype(np.float64)
              * post_bin_scale.reshape(-1).astype(np.float64)).astype(np.float32)
    c1_all = (np.float64(0.5) * sigma
              * final_scale.astype(np.float64)).astype(np.float32)
    c0 = np.ascontiguousarray(c0_all.reshape(NB_O, P).T)
    c1 = np.ascontiguousarray(c1_all.reshape(NB_O, P).T)
    fb = np.ascontiguousarray(
        np.broadcast_to(final_bias.astype(np.float32), (P, D_OUT)))
    return inv_se, c0, c1, fb


def _run_spmd(nc, maps):
    """Execute with retry: axon-tunneled devices can transiently fail."""
    from concourse.bass_utils import run_bass_kernel_spmd
    for attempt in range(3):
        try:
            return run_bass_kernel_spmd(nc, maps, core_ids=list(range(N_CORES)))
        except Exception:  # noqa: BLE001 - retrying device-side faults
            if attempt == 2:
                raise
            import gc
            import time as _time
            gc.collect()
            try:
                import jax
                jax.clear_caches()
                import jax.extend as _jex
                _jex.backend.clear_backends()
            except Exception:
                pass
            _time.sleep(10)


def prepare(x, weight, post_bin_scale, final_scale, final_bias, running_max,
            sprinkle_mask):
    """Build (compile) the module and the per-core input maps."""
    x = np.asarray(x, dtype=np.float32)
    weight = np.ascontiguousarray(np.asarray(weight, dtype=np.float32))
    mask = np.asarray(sprinkle_mask)
    pbs = np.asarray(post_bin_scale, dtype=np.float32)
    fs = np.asarray(final_scale, dtype=np.float32)
    fb = np.asarray(final_bias, dtype=np.float32)
    rm = float(np.asarray(running_max))

    if np.all(pbs.reshape(-1) == pbs.reshape(-1)[0]):
        inv_se, c0s, c1s, fscol, fbcol = _fast_consts(pbs, fs, fb, rm)
        nc = _build_fast(inv_se, c0s, c1s)
        maps = _fast_in_maps(x, weight, mask, fscol, fbcol)
        fast = True
    else:
        inv_se, c0, c1, fbb = _general_consts(pbs, fs, fb, rm)
        nc = _build_general(inv_se)
        maps = _general_in_maps(
            x, weight, np.ascontiguousarray(mask).view(np.uint8), c0, c1, fbb)
        fast = False
    return nc, maps, fast


def kernel(x, weight, post_bin_scale, final_scale, final_bias, running_max,
           sprinkle_mask):
    nc, maps, fast = prepare(x, weight, post_bin_scale, final_scale,
                             final_bias, running_max, sprinkle_mask)
    res = _run_spmd(nc, maps)
    if fast:
        out = np.concatenate(
            [np.asarray(res.results[c]["out"]).astype(np.float32).T
             for c in range(N_CORES)], axis=0)
    else:
        out = np.concatenate(
            [res.results[c]["out"] for c in range(N_CORES)], axis=0)
    return np.ascontiguousarray(out)


# revision 18
# speedup vs baseline: 1.3844x; 1.1594x over previous
"""BitLinear (quantized-activation, binarized-sprinkled-weight linear) Trainium2 kernel.

Data-parallel over the token dim N across 8 NeuronCores.

Fast path (post_bin_scale uniform, which holds for this problem's inputs):
  * all operands are laid out HOST-side so the contraction dim i lands on
    SBUF partitions with no device transposes at all:
       xT [IN, N_SHARD] f32 (per core),  wT [IN, OUT] f32,  mT [IN, OUT] u16
  * w/x are DMA-cast to bf16 on load (gpsimd SWDGE); mask is u16 so every
    DVE operand is 2-byte -> double-pumped custom DVE ops
  * one fused DVE op per operand:
       xb = t + clip(round(t), +-127),  t = x*inv_se          (bf16)
       W2 = m ? sign(w)*C0 : w*C1 + sign(w)*C0*0.5            (bf16)
    with C0 = sigma*pbs, C1 = 0.5*sigma global scalars (sigma = 0.5*(s+eps)),
    so that  out[n,o] = (xb @ W2^T)*fs[o] + fb[o]
  * matmuls produce PSUM tiles [o=128, n=512] (o on partitions), 16 k-steps
    each, issued in waves of 8 interleaved k-outer so the PE can consume
    operand blocks as the DVE produces them
  * epilogue on the Activation engine: Identity with per-partition scale=fs,
    bias=fb, writing bf16; stores go out transposed [OUT, N_SHARD] and the
    host transposes back (layout-only work).

General path (non-uniform post_bin_scale): previous per-partition-constant
implementation, kept verbatim as a fallback.
"""

import numpy as np

N_CORES = 8
N_TOK, D_IN, D_OUT = 8192, 2048, 2048
N_SHARD = N_TOK // N_CORES          # 1024 rows of x per core
P = 128
NJ = N_SHARD // P                   # 8 n-blocks per core
NB_I = D_IN // P                    # 16 i-blocks (contraction)
NB_O = D_OUT // P                   # 16 o-blocks
OT = 512                            # o-tile (one PSUM bank)
NT = D_OUT // OT                    # 4 o-tiles
OB_PER_T = OT // P                  # 4 o-blocks per o-tile

MM_NT = N_SHARD // 512              # fast path: 2 moving-dim tiles of 512

QMAX = 127.0
EPS = 1e-6
MAGIC = 12582912.0                  # 1.5 * 2**23: fp32 RNE round-to-int trick

_CACHE = {}


def _register_ops():
    """Define the two fused DVE ops (idempotent)."""
    from concourse import dve_ops
    from concourse.dve_spec import (
        Spec, Src0, Src1, C0, C1, C2, Zero, select, minn, maxx, lower, _has_src1,
    )
    from concourse.dve_uop import DveOpSpec

    def register(name, spec):
        for op in dve_ops.OPS:
            if op.name == name:
                return op
        ver = "v3"
        tmp = DveOpSpec(name=name, opcode=0, uops=lower(spec, ver=ver),
                        rd1_en=_has_src1(spec))
        op = dve_ops.DveOp(name, spec, subdim=False,
                           uops_sha={ver: tmp.sha(ver)})
        dve_ops.OPS.append(op)
        dve_ops._SUB_OPCODE_FOR_NAME[name] = (
            max(dve_ops._SUB_OPCODE_FOR_NAME.values()) + 1)
        dve_ops.CUSTOM_DVE_SPECS[name] = spec
        return op

    # out = t + clip(round(t), +-imm2), t = x*s0   (s1 = MAGIC)
    _t = Src0 * C0
    _r = (_t + C1) - C1
    _rc = minn(maxx(_r, Zero - C2), C2)
    xprep = register("XPREP_BITLIN", Spec(
        body=_t + _rc,
        reference=lambda in0, in1, s0, s1, imm2: (
            (lambda t: t + np.clip(np.round(t), -imm2, imm2))(
                in0.astype(np.float32) * s0)),
    ))

    # h = select(w>=0, s0, -s0); out = select(m>0, h, w*s1 + h*imm2)
    _h = select(Src0 >= Zero, C0, Zero - C0)
    wprep = register("WPREP_BITLIN", Spec(
        body=select(Src1 > Zero, _h, Src0 * C1 + _h * C2),
        reference=lambda in0, in1, s0, s1, imm2: (
            (lambda h: np.where(in1 > 0, h,
                                in0.astype(np.float32) * s1 + h * imm2))(
                np.where(in0 >= 0, s0, -s0))),
    ))

    # two-constant variant (no imm2 -> allows 2-free-dim src1):
    # hh = select(w>=0, s0, -s0); out = select(m>0, hh+hh, w*s1 + hh)
    # with s0 = 0.5*C0 so that hh+hh = sign(w)*C0.
    _hh = select(Src0 >= Zero, C0, Zero - C0)
    wprep2 = register("WPREP2_BITLIN", Spec(
        body=select(Src1 > Zero, _hh + _hh, Src0 * C1 + _hh),
        reference=lambda in0, in1, s0, s1, imm2: (
            (lambda h: np.where(in1 > 0, h + h,
                                in0.astype(np.float32) * s1 + h))(
                np.where(in0 >= 0, s0, -s0))),
    ))
    return xprep, wprep, wprep2


def _build_fast(inv_se, c0s, c1s):
    """Fast-path per-core Bass module (uniform post_bin_scale).

    inv_se, c0s, c1s are global scalars baked into the DVE ops.
    """
    key = ("fast", float(inv_se), float(c0s), float(c1s))
    if key in _CACHE:
        return _CACHE[key]

    import concourse.mybir as mybir
    import concourse.tile as tile
    from concourse import bacc

    xprep, _, wprep2 = _register_ops()

    nc = bacc.Bacc(None, target_bir_lowering=False)
    bf16 = mybir.dt.bfloat16
    f32 = mybir.dt.float32
    ident = mybir.ActivationFunctionType.Identity

    x_in = nc.dram_tensor("x", [D_IN, N_SHARD], f32, kind="ExternalInput")
    w_in = nc.dram_tensor("w", [D_IN, D_OUT], f32, kind="ExternalInput")
    m_in = nc.dram_tensor("m", [D_IN, D_OUT], mybir.dt.uint8, kind="ExternalInput")
    fs_in = nc.dram_tensor("fs", [P, NB_O], f32, kind="ExternalInput")
    fb_in = nc.dram_tensor("fb", [P, NB_O], f32, kind="ExternalInput")
    out_o = nc.dram_tensor("out", [D_OUT, N_SHARD], bf16, kind="ExternalOutput")

    KH = NB_I // 2                  # contraction split: 8 + 8 k-steps
    tiles = [(ob, nt) for nt in range(MM_NT) for ob in range(NB_O)]
    waves = [tiles[w0:w0 + 8] for w0 in range(0, len(tiles), 8)]

    with tile.TileContext(nc) as tc:
        with (
            tc.tile_pool(name="persist", bufs=1) as persist,
            tc.tile_pool(name="wlp", bufs=4) as wlp,
            tc.tile_pool(name="xlp", bufs=4) as xlp,
            tc.tile_pool(name="ostage", bufs=8) as ostage,
            tc.tile_pool(name="psum", bufs=8, space="PSUM") as psum,
        ):
            w2 = persist.tile([P, NB_I, D_OUT], bf16, tag="w2")   # [i_in, ib, o]
            xb = persist.tile([P, NB_I, N_SHARD], bf16, tag="xb")  # [i_in, ib, n]
            # bf16 partial sums for the first contraction half (PSUM is only
            # 8 banks; parking partials in SBUF lets all 32 output tiles
            # overlap the operand-prep phase)
            parts = persist.tile([P, len(tiles), 512], bf16, tag="parts")
            fs_sb = persist.tile([P, NB_O], f32, tag="fs")
            fb_sb = persist.tile([P, NB_O], f32, tag="fb")

            nc.sync.dma_start(fs_sb[:], fs_in[:])
            nc.sync.dma_start(fb_sb[:], fb_in[:])

            # Operand prep in dependency order. Phase-1 wave 1 (nt=0, ob 0-7)
            # consumes one (Xa, Wa) half-block pair per 1.7 us; emitting
            # exactly those halves first makes DVE production match PE
            # consumption, eliminating wave-1 pacing stalls. Remaining halves
            # follow in the order later waves need them.
            XHF, WHF = N_SHARD // 2, D_OUT // 2

            def x_op(ib, lo, sz):
                xt = xlp.tile([P, sz], bf16, name="xt", tag="x_bf16")
                nc.gpsimd.dma_start(xt[:], x_in[ib * P:(ib + 1) * P, lo:lo + sz])
                nc.vector._custom_dve(
                    xprep, out=xb[:, ib, lo:lo + sz], in0=xt[:],
                    s0=float(inv_se), s1=MAGIC, imm2=QMAX)

            def w_op(ib, lo, sz):
                mt = wlp.tile([P, sz], mybir.dt.uint8, name="mt", tag="m_u8")
                nc.sync.dma_start(mt[:], m_in[ib * P:(ib + 1) * P, lo:lo + sz])
                wt = wlp.tile([P, sz], bf16, name="wt", tag="w_bf16")
                nc.gpsimd.dma_start(wt[:], w_in[ib * P:(ib + 1) * P, lo:lo + sz])
                nc.vector._custom_dve(
                    wprep2, out=w2[:, ib, lo:lo + sz], in0=wt[:], in1=mt[:],
                    s0=float(0.5 * c0s), s1=float(c1s))

            for ib in range(KH):                 # wave 1 deps, paced 1:1
                x_op(ib, 0, XHF)
                w_op(ib, 0, WHF)
            for ib in range(KH):                 # wave 2 deps (nt0, ob 8-15)
                w_op(ib, WHF, WHF)
            for ib in range(KH):                 # waves 3/4 deps (nt1)
                x_op(ib, XHF, XHF)
            for ib in range(KH, NB_I):           # second contraction half
                x_op(ib, 0, N_SHARD)
                w_op(ib, 0, D_OUT)

            def mm(ps, ob, nt, ib, start, stop):
                nc.tensor.matmul(
                    ps[:],
                    w2[:, ib, ob * P:(ob + 1) * P],
                    xb[:, ib, nt * 512:(nt + 1) * 512],
                    start=start, stop=stop, skip_group_check=True)

            # phase 1 (overlaps prep): first contraction half for every tile,
            # k-outer within each 8-tile wave; partials parked in SBUF bf16.
            # The last wave's final 4 tiles keep their banks: phase 2
            # continues them in place while the first reloads pipeline in.
            held = []
            for wi, wave in enumerate(waves):
                pss = [psum.tile([P, 512], f32, name="ps", tag="ps")
                       for _ in wave]
                for ib in range(KH):
                    for ps, (ob, nt) in zip(pss, wave):
                        mm(ps, ob, nt, ib, ib == 0, ib == KH - 1)
                if wi == len(waves) - 1:
                    for k in range(4):
                        nc.scalar.copy(parts[:, wi * 8 + k, :], pss[k][:])
                    held = pss[4:]
                else:
                    for k, ps in enumerate(pss):
                        nc.scalar.copy(parts[:, wi * 8 + k, :], ps[:])

            def epilogue(ps, ob, nt):
                osb = ostage.tile([P, 512], bf16, tag="osb")
                nc.scalar.activation(
                    osb[:], ps[:], ident,
                    bias=fb_sb[:, ob:ob + 1], scale=fs_sb[:, ob:ob + 1])
                nc.sync.dma_start(
                    out_o[ob * P:(ob + 1) * P, nt * 512:(nt + 1) * 512],
                    osb[:])

            # phase 2: finish the 4 held tiles first (banks already hot) while
            # the first reload wave pipelines in on the Act queue; then parked
            # tiles in small waves, reloads running ahead of epilogues.
            parked = tiles[:28]
            held_tiles = tiles[28:]
            sizes = [4, 4, 4, 4, 4, 4, 2, 1, 1]
            p2, pos = [], 0
            for sz in sizes:
                p2.append(list(enumerate(parked[pos:pos + sz], start=pos)))
                pos += sz

            def load_wave(wi):
                pss = [psum.tile([P, 512], f32, name="ps", tag="ps")
                       for _ in p2[wi]]
                for ps, (idx, _) in zip(pss, p2[wi]):
                    nc.scalar.copy(ps[:], parts[:, idx, :])
                return pss

            live = {0: load_wave(0)}
            for ib in range(KH, NB_I):
                for ps, (ob, nt) in zip(held, held_tiles):
                    mm(ps, ob, nt, ib, False, ib == NB_I - 1)
            for ps, (ob, nt) in zip(held, held_tiles):
                epilogue(ps, ob, nt)
            live[1] = load_wave(1)

            for wi, wave in enumerate(p2):
                pss = live.pop(wi)
                for ib in range(KH, NB_I):
                    for ps, (_, (ob, nt)) in zip(pss, wave):
                        mm(ps, ob, nt, ib, False, ib == NB_I - 1)
                for ps, (_, (ob, nt)) in zip(pss, wave):
                    epilogue(ps, ob, nt)
                if wi + 2 < len(p2):
                    live[wi + 2] = load_wave(wi + 2)

    nc.compile()
    _CACHE[key] = nc
    return nc


def _fast_consts(post_bin_scale, final_scale, final_bias, running_max):
    s = np.float32(running_max) / np.float32(QMAX)
    inv_se = np.float32(1.0) / (s + np.float32(EPS))
    sigma = np.float64(0.5) * (np.float64(s) + np.float64(EPS))
    pbs0 = np.float64(post_bin_scale.reshape(-1)[0])
    c0s = np.float32(sigma * pbs0)
    c1s = np.float32(np.float64(0.5) * sigma)
    fscol = np.ascontiguousarray(
        final_scale.astype(np.float32).reshape(NB_O, P).T)
    fbcol = np.ascontiguousarray(
        final_bias.astype(np.float32).reshape(NB_O, P).T)
    return inv_se, c0s, c1s, fscol, fbcol


def _fast_in_maps(x, weight, mask, fscol, fbcol):
    wT = np.ascontiguousarray(weight.T)
    mT = np.ascontiguousarray(mask.T).view(np.uint8)
    maps = []
    for c in range(N_CORES):
        maps.append({
            "x": np.ascontiguousarray(x[c * N_SHARD:(c + 1) * N_SHARD].T),
            "w": wT,
            "m": mT,
            "fs": fscol,
            "fb": fbcol,
        })
    return maps


# ---------------------------------------------------------------------------
# general fallback (non-uniform post_bin_scale): previous implementation
# ---------------------------------------------------------------------------

def _build_general(inv_se):
    key = ("nc", float(inv_se))
    if key in _CACHE:
        return _CACHE[key]

    import concourse.mybir as mybir
    import concourse.tile as tile
    from concourse import bacc

    xprep, wprep, _ = _register_ops()

    nc = bacc.Bacc(None, target_bir_lowering=False)
    bf16 = mybir.dt.bfloat16
    f32 = mybir.dt.float32

    x_in = nc.dram_tensor("x", [N_SHARD, D_IN], f32, kind="ExternalInput")
    w_in = nc.dram_tensor("w", [D_OUT, D_IN], f32, kind="ExternalInput")
    m_in = nc.dram_tensor("m", [D_OUT, D_IN], mybir.dt.uint8, kind="ExternalInput")
    c0_in = nc.dram_tensor("c0", [P, NB_O], f32, kind="ExternalInput")
    c1_in = nc.dram_tensor("c1", [P, NB_O], f32, kind="ExternalInput")
    fb_in = nc.dram_tensor("fb", [P, D_OUT], f32, kind="ExternalInput")
    out_o = nc.dram_tensor("out", [N_SHARD, D_OUT], f32, kind="ExternalOutput")

    from concourse.masks import make_identity

    with tile.TileContext(nc) as tc:
        with (
            tc.tile_pool(name="persist", bufs=1) as persist,
            tc.tile_pool(name="wlp", bufs=4) as wlp,
            tc.tile_pool(name="wpp", bufs=4) as wpp,
            tc.tile_pool(name="xlp", bufs=4) as xlp,
            tc.tile_pool(name="xbp", bufs=4) as xbp,
            tc.tile_pool(name="ostage", bufs=7) as ostage,
            tc.tile_pool(name="psum", bufs=6, space="PSUM") as psum,
            tc.tile_pool(name="tpsum", bufs=2, space="PSUM") as tpsum,
        ):
            wT = persist.tile([P, NB_O, NB_I, P], bf16, tag="wT")
            xqT = persist.tile([P, NJ, NB_I, P], bf16, tag="xqT")
            c0_sb = persist.tile([P, NB_O], f32, tag="c0")
            c1_sb = persist.tile([P, NB_O], f32, tag="c1")
            fb_sb = persist.tile([P, D_OUT], f32, tag="fb")
            ident = persist.tile([P, P], bf16, tag="ident")

            nc.sync.dma_start(fb_sb[:], fb_in[:])
            nc.sync.dma_start(c0_sb[:], c0_in[:])
            nc.sync.dma_start(c1_sb[:], c1_in[:])
            make_identity(nc, ident[:])

            def w_block(ob):
                wt = wlp.tile([P, D_IN], bf16, tag="w_bf16")
                mt = wlp.tile([P, D_IN], mybir.dt.uint8, tag="m_u8")
                nc.gpsimd.dma_start(wt[:], w_in[ob * P:(ob + 1) * P, :])
                nc.scalar.dma_start(mt[:], m_in[ob * P:(ob + 1) * P, :])
                w2 = wpp.tile([P, D_IN], bf16, tag="w2")
                nc.vector._custom_dve(
                    wprep, out=w2[:], in0=wt[:], in1=mt[:],
                    s0=c0_sb[:, ob:ob + 1], s1=c1_sb[:, ob:ob + 1], imm2=0.5)
                nc.sync.dma_start_transpose(wT[:, ob], w2[:])

            def x_block(j):
                xt = xlp.tile([P, D_IN], f32, tag="x_f32")
                nc.sync.dma_start(xt[:], x_in[j * P:(j + 1) * P, :])
                xb = xbp.tile([P, D_IN], bf16, tag="xb")
                nc.vector._custom_dve(
                    xprep, out=xb[:], in0=xt[:],
                    s0=float(inv_se), s1=MAGIC, imm2=QMAX)
                for b in range(NB_I):
                    tp = tpsum.tile([P, P], bf16, tag="xtp")
                    nc.tensor.transpose(tp[:], xb[:, b * P:(b + 1) * P], ident[:])
                    nc.scalar.copy(xqT[:, j, b, :], tp[:])

            for ob in range(OB_PER_T):
                w_block(ob)
            for j in range(NJ):
                x_block(j)
            for ob in range(OB_PER_T, NB_O):
                w_block(ob)

            for t in range(NT):
                for j in range(NJ):
                    ps = psum.tile([P, OT], f32, tag="ps")
                    for b in range(NB_I):
                        nc.tensor.matmul(
                            ps[:],
                            xqT[:, j, b, :],
                            wT[:, t * OB_PER_T:(t + 1) * OB_PER_T, b, :],
                            start=(b == 0), stop=(b == NB_I - 1))
                    osb = ostage.tile([P, OT], f32, tag="osb")
                    nc.vector.tensor_add(
                        osb[:], ps[:], fb_sb[:, t * OT:(t + 1) * OT])
                    nc.scalar.dma_start(
                        out_o[j * P:(j + 1) * P, t * OT:(t + 1) * OT], osb[:])

    nc.compile()
    _CACHE[key] = nc
    return nc


def _general_in_maps(x, weight, mask_u8, c0, c1, fb):
    maps = []
    for c in range(N_CORES):
        maps.append({
            "x": np.ascontiguousarray(x[c * N_SHARD:(c + 1) * N_SHARD]),
            "w": weight,
            "m": mask_u8,
            "c0": c0,
            "c1": c1,
            "fb": fb,
        })
    return maps


def _general_consts(post_bin_scale, final_scale, final_bias, running_max):
    s = np.float32(running_max) / np.float32(QMAX)
    inv_se = np.float32(1.0) / (s + np.float32(EPS))
    sigma = np.float64(0.5) * (np.float64(s) + np.float64(EPS))
    c0_all = (sigma * final_scale.astype(np.float64)
              * post_bin_scale.reshape(-1).astype(np.float64)).astype(np.float32)
    c1_all = (np.float64(0.5) * sigma
              * final_scale.astype(np.float64)).astype(np.float32)
    c0 = np.ascontiguousarray(c0_all.reshape(NB_O, P).T)
    c1 = np.ascontiguousarray(c1_all.reshape(NB_O, P).T)
    fb = np.ascontiguousarray(
        np.broadcast_to(final_bias.astype(np.float32), (P, D_OUT)))
    return inv_se, c0, c1, fb


def _run_spmd(nc, maps):
    """Execute with retry: axon-tunneled devices can transiently fail."""
    from concourse.bass_utils import run_bass_kernel_spmd
    for attempt in range(3):
        try:
            return run_bass_kernel_spmd(nc, maps, core_ids=list(range(N_CORES)))
        except Exception:  # noqa: BLE001 - retrying device-side faults
            if attempt == 2:
                raise
            import gc
            import time as _time
            gc.collect()
            try:
                import jax
                jax.clear_caches()
                import jax.extend as _jex
                _jex.backend.clear_backends()
            except Exception:
                pass
            _time.sleep(10)


def prepare(x, weight, post_bin_scale, final_scale, final_bias, running_max,
            sprinkle_mask):
    """Build (compile) the module and the per-core input maps."""
    x = np.asarray(x, dtype=np.float32)
    weight = np.ascontiguousarray(np.asarray(weight, dtype=np.float32))
    mask = np.asarray(sprinkle_mask)
    pbs = np.asarray(post_bin_scale, dtype=np.float32)
    fs = np.asarray(final_scale, dtype=np.float32)
    fb = np.asarray(final_bias, dtype=np.float32)
    rm = float(np.asarray(running_max))

    if np.all(pbs.reshape(-1) == pbs.reshape(-1)[0]):
        inv_se, c0s, c1s, fscol, fbcol = _fast_consts(pbs, fs, fb, rm)
        nc = _build_fast(inv_se, c0s, c1s)
        maps = _fast_in_maps(x, weight, mask, fscol, fbcol)
        fast = True
    else:
        inv_se, c0, c1, fbb = _general_consts(pbs, fs, fb, rm)
        nc = _build_general(inv_se)
        maps = _general_in_maps(
            x, weight, np.ascontiguousarray(mask).view(np.uint8), c0, c1, fbb)
        fast = False
    return nc, maps, fast


def kernel(x, weight, post_bin_scale, final_scale, final_bias, running_max,
           sprinkle_mask):
    nc, maps, fast = prepare(x, weight, post_bin_scale, final_scale,
                             final_bias, running_max, sprinkle_mask)
    res = _run_spmd(nc, maps)
    if fast:
        out = np.concatenate(
            [np.asarray(res.results[c]["out"]).astype(np.float32).T
             for c in range(N_CORES)], axis=0)
    else:
        out = np.concatenate(
            [res.results[c]["out"] for c in range(N_CORES)], axis=0)
    return np.ascontiguousarray(out)


# revision 26
# speedup vs baseline: 1.4549x; 1.0509x over previous
"""BitLinear (quantized-activation, binarized-sprinkled-weight linear) Trainium2 kernel.

Data-parallel over the token dim N across 8 NeuronCores.

Fast path (post_bin_scale uniform, which holds for this problem's inputs):
  * all operands are laid out HOST-side (pure transposes/views) so the
    contraction dim i lands on SBUF partitions with no device transposes:
       xT [IN, N_SHARD] f32 (per core),  wT [IN, OUT] f32,  mT [IN, OUT] u8
  * w/x are DMA-cast to bf16 on load (gpsimd SWDGE); one fused DVE op per
    operand block:
       xb = t + clip(round(t), +-127),  t = x*inv_se          (bf16)
       W2 = m ? sign(w)*C0 : w*C1 + sign(w)*C0*0.5            (bf16)
    with C0 = sigma*pbs, C1 = 0.5*sigma global scalars (sigma = 0.5*(s+eps)),
    so that  out[n,o] = (xb @ W2^T)*fs[o] + fb[o]
  * prep is emitted in dependency order at half-block granularity so DVE
    production (the prep bottleneck, custom ops run 1x) matches the PE's
    PSUM-capacity-limited consumption exactly -- no matmul pacing stalls
  * matmuls produce PSUM tiles [o=128, n=512] (o on partitions), 16 k-steps.
    PSUM only holds 8 such tiles, so the contraction is split 8+8: during
    prep, every output tile's first half accumulates in waves of 8 and is
    parked in SBUF as bf16 partials; after prep the halves are reloaded
    (Act engine, software-pipelined two waves deep) and finished
  * epilogue on the Activation engine: Identity with per-partition scale=fs,
    bias=fb, writing bf16; stores go out transposed [OUT, N_SHARD] and the
    host transposes back / upcasts (layout-only work).

General path (non-uniform post_bin_scale): previous per-partition-constant
implementation, kept verbatim as a fallback.
"""

import numpy as np

N_CORES = 8
N_TOK, D_IN, D_OUT = 8192, 2048, 2048
N_SHARD = N_TOK // N_CORES          # 1024 rows of x per core
P = 128
NJ = N_SHARD // P                   # 8 n-blocks per core
NB_I = D_IN // P                    # 16 i-blocks (contraction)
NB_O = D_OUT // P                   # 16 o-blocks
OT = 512                            # o-tile (one PSUM bank)
NT = D_OUT // OT                    # 4 o-tiles
OB_PER_T = OT // P                  # 4 o-blocks per o-tile

MM_NT = N_SHARD // 512              # fast path: 2 moving-dim tiles of 512

QMAX = 127.0
EPS = 1e-6
MAGIC = 12582912.0                  # 1.5 * 2**23: fp32 RNE round-to-int trick

_CACHE = {}


def _register_ops():
    """Define the two fused DVE ops (idempotent)."""
    from concourse import dve_ops
    from concourse.dve_spec import (
        Spec, Src0, Src1, C0, C1, C2, Zero, select, minn, maxx, lower, _has_src1,
    )
    from concourse.dve_uop import DveOpSpec

    def register(name, spec):
        for op in dve_ops.OPS:
            if op.name == name:
                return op
        ver = "v3"
        tmp = DveOpSpec(name=name, opcode=0, uops=lower(spec, ver=ver),
                        rd1_en=_has_src1(spec))
        op = dve_ops.DveOp(name, spec, subdim=False,
                           uops_sha={ver: tmp.sha(ver)})
        dve_ops.OPS.append(op)
        dve_ops._SUB_OPCODE_FOR_NAME[name] = (
            max(dve_ops._SUB_OPCODE_FOR_NAME.values()) + 1)
        dve_ops.CUSTOM_DVE_SPECS[name] = spec
        return op

    # out = t + clip(round(t), +-imm2), t = x*s0   (s1 = MAGIC)
    _t = Src0 * C0
    _r = (_t + C1) - C1
    _rc = minn(maxx(_r, Zero - C2), C2)
    xprep = register("XPREP_BITLIN", Spec(
        body=_t + _rc,
        reference=lambda in0, in1, s0, s1, imm2: (
            (lambda t: t + np.clip(np.round(t), -imm2, imm2))(
                in0.astype(np.float32) * s0)),
    ))

    # h = select(w>=0, s0, -s0); out = select(m>0, h, w*s1 + h*imm2)
    _h = select(Src0 >= Zero, C0, Zero - C0)
    wprep = register("WPREP_BITLIN", Spec(
        body=select(Src1 > Zero, _h, Src0 * C1 + _h * C2),
        reference=lambda in0, in1, s0, s1, imm2: (
            (lambda h: np.where(in1 > 0, h,
                                in0.astype(np.float32) * s1 + h * imm2))(
                np.where(in0 >= 0, s0, -s0))),
    ))

    # two-constant variant (no imm2 -> allows 2-free-dim src1):
    # hh = select(w>=0, s0, -s0); out = select(m>0, hh+hh, w*s1 + hh)
    # with s0 = 0.5*C0 so that hh+hh = sign(w)*C0.
    _hh = select(Src0 >= Zero, C0, Zero - C0)
    wprep2 = register("WPREP2_BITLIN", Spec(
        body=select(Src1 > Zero, _hh + _hh, Src0 * C1 + _hh),
        reference=lambda in0, in1, s0, s1, imm2: (
            (lambda h: np.where(in1 > 0, h + h,
                                in0.astype(np.float32) * s1 + h))(
                np.where(in0 >= 0, s0, -s0))),
    ))
    return xprep, wprep, wprep2


def _build_fast(inv_se, c0s, c1s):
    """Fast-path per-core Bass module (uniform post_bin_scale).

    inv_se, c0s, c1s are global scalars baked into the DVE ops.
    """
    key = ("fast", float(inv_se), float(c0s), float(c1s))
    if key in _CACHE:
        return _CACHE[key]

    import concourse.mybir as mybir
    import concourse.tile as tile
    from concourse import bacc

    xprep, _, wprep2 = _register_ops()

    nc = bacc.Bacc(None, target_bir_lowering=False)
    bf16 = mybir.dt.bfloat16
    f32 = mybir.dt.float32
    ident = mybir.ActivationFunctionType.Identity

    x_in = nc.dram_tensor("x", [NB_I, P, N_SHARD], f32, kind="ExternalInput")
    w_in = nc.dram_tensor("w", [NB_I, P, D_OUT], f32, kind="ExternalInput")
    m_in = nc.dram_tensor("m", [NB_I, P, D_OUT], mybir.dt.uint8,
                          kind="ExternalInput")
    fs_in = nc.dram_tensor("fs", [P, NB_O], f32, kind="ExternalInput")
    fb_in = nc.dram_tensor("fb", [P, NB_O], f32, kind="ExternalInput")
    out_o = nc.dram_tensor("out", [D_OUT, N_SHARD], bf16, kind="ExternalOutput")

    KH = NB_I // 2                  # contraction split: 8 + 8 k-steps
    # wave order matches operand production order: (nt0,ob0-7) needs only the
    # Xa/Wa halves, (nt1,ob0-7) adds Xb, (nt0,ob8-15) adds Wb, (nt1,ob8-15)
    # needs everything -- so no wave ever waits on prep at a phase boundary
    tiles = ([(ob, 0) for ob in range(8)] + [(ob, 1) for ob in range(8)]
             + [(ob, 0) for ob in range(8, 16)]
             + [(ob, 1) for ob in range(8, 16)])
    waves = [tiles[w0:w0 + 8] for w0 in range(0, len(tiles), 8)]

    with tile.TileContext(nc) as tc:
        with (
            tc.tile_pool(name="persist", bufs=1) as persist,
            tc.tile_pool(name="wlp", bufs=6) as wlp,
            tc.tile_pool(name="mlp", bufs=6) as mlp,
            tc.tile_pool(name="xlp", bufs=5) as xlp,
            tc.tile_pool(name="ostage", bufs=8) as ostage,
            tc.tile_pool(name="psum", bufs=8, space="PSUM") as psum,
        ):
            w2 = persist.tile([P, NB_I, D_OUT], bf16, tag="w2")   # [i_in, ib, o]
            xb = persist.tile([P, NB_I, N_SHARD], bf16, tag="xb")  # [i_in, ib, n]
            # bf16 partial sums for the first contraction half (PSUM is only
            # 8 banks; parking partials in SBUF lets all 32 output tiles
            # overlap the operand-prep phase)
            parts = persist.tile([P, len(tiles), 512], bf16, tag="parts")
            fs_sb = persist.tile([P, NB_O], f32, tag="fs")
            fb_sb = persist.tile([P, NB_O], f32, tag="fb")

            # PE warm-up source: a zeroed bf16 tile for dummy matmuls that
            # keep the tensor engine busy through the prep-latency window so
            # real matmuls start at full p-state.
            warm = persist.tile([P, 512], bf16, tag="warm")
            nc.vector.memset(warm[:], 0.0)

            # Operand prep in dependency order. Phase-1 wave 1 (nt=0, ob 0-7)
            # consumes one (Xa, Wa) half-block pair per 1.7 us; emitting
            # exactly those halves first makes DVE production match PE
            # consumption, eliminating wave-1 pacing stalls. Remaining halves
            # follow in the order later waves need them.
            XHF, WHF = N_SHARD // 2, D_OUT // 2

            def x_op(ib, lo, sz, hwdge=False):
                # hwdge=True: plain f32 load on the sync HWDGE ring. Used in
                # the pacing-critical first phase so the Pool engine's SWDGE
                # desc-gen (994ns fixed per DMA) only carries the w loads and
                # operand production keeps up with wave-1 consumption.
                if hwdge:
                    xt = xlp.tile([P, sz], f32, name="xt", tag="x_f32")
                    nc.sync.dma_start(xt[:],
                                      x_in[ib, :, lo:lo + sz])
                else:
                    xt = xlp.tile([P, sz], bf16, name="xt", tag="x_bf16")
                    nc.gpsimd.dma_start(xt[:],
                                        x_in[ib, :, lo:lo + sz])
                nc.vector._custom_dve(
                    xprep, out=xb[:, ib, lo:lo + sz], in0=xt[:],
                    s0=float(inv_se), s1=MAGIC, imm2=QMAX)

            def w_op(ib, lo, sz):
                mt = mlp.tile([P, sz], mybir.dt.uint8, name="mt", tag="m_u8")
                nc.sync.dma_start(mt[:], m_in[ib, :, lo:lo + sz])
                wt = wlp.tile([P, sz], bf16, name="wt", tag="w_bf16")
                nc.gpsimd.dma_start(wt[:], w_in[ib, :, lo:lo + sz])
                nc.vector._custom_dve(
                    wprep2, out=w2[:, ib, lo:lo + sz], in0=wt[:], in1=mt[:],
                    s0=float(0.5 * c0s), s1=float(c1s))

            # wave 1 deps, paced 1:1. xa halves are loaded two i-blocks per
            # SWDGE DMA (one desc-gen instead of two) so Pool desc-gen plus
            # the w loads stay under the DVE production floor.
            xa2 = {}

            def xa2_load(ibp):
                t2 = xlp.tile([P, 2, XHF], bf16, name="t2", tag="x_bf16p")
                nc.gpsimd.dma_start(
                    t2[:], x_in[2 * ibp:2 * ibp + 2, :, 0:XHF]
                    .transpose([1, 0, 2]))
                xa2[ibp] = t2

            xa2_load(0)
            for ib in range(KH):
                nc.vector._custom_dve(
                    xprep, out=xb[:, ib, 0:XHF], in0=xa2[ib // 2][:, ib % 2, :],
                    s0=float(inv_se), s1=MAGIC, imm2=QMAX)
                w_op(ib, 0, WHF)
                if ib % 2 == 0 and ib // 2 + 1 < KH // 2:
                    xa2_load(ib // 2 + 1)
            for ib in range(KH):                 # wave 2 deps (nt1, ob 0-7)
                x_op(ib, XHF, XHF)
            for ib in range(KH):                 # wave 3 deps (nt0, ob 8-15)
                w_op(ib, WHF, WHF)
            for ib in range(KH, NB_I):           # second contraction half
                x_op(ib, 0, N_SHARD)
                w_op(ib, 0, D_OUT)

            # epilogue constants (needed only after the first full tiles
            # finish, so loaded after the prep stream is underway)
            nc.scalar.dma_start(fs_sb[:], fs_in[:])
            nc.scalar.dma_start(fb_sb[:], fb_in[:])

            def mm(ps, ob, nt, ib, start, stop):
                nc.tensor.matmul(
                    ps[:],
                    w2[:, ib, ob * P:(ob + 1) * P],
                    xb[:, ib, nt * 512:(nt + 1) * 512],
                    start=start, stop=stop, skip_group_check=True)

            # phase 1 (overlaps prep): first contraction half for every tile,
            # k-outer within each 8-tile wave; partials parked in SBUF bf16.
            # The last wave's final 4 tiles keep their banks: phase 2
            # continues them in place while the first reloads pipeline in.
            held = []
            for wi, wave in enumerate(waves):
                pss = [psum.tile([P, 512], f32, name="ps", tag="ps")
                       for _ in wave]
                if wi == 0:
                    # dummies into pss[0]; the real ib-0 matmul resets it
                    # (start=True), so only timing is affected.
                    for _ in range(7):
                        nc.tensor.matmul(pss[0][:], warm[:, 0:P], warm[:],
                                         start=True, stop=True,
                                         skip_group_check=True)
                for ib in range(KH):
                    for ps, (ob, nt) in zip(pss, wave):
                        mm(ps, ob, nt, ib, ib == 0, ib == KH - 1)
                if wi == len(waves) - 1:
                    for k in range(4):
                        nc.scalar.copy(parts[:, wi * 8 + k, :], pss[k][:])
                    held = pss[4:]
                else:
                    for k, ps in enumerate(pss):
                        nc.scalar.copy(parts[:, wi * 8 + k, :], ps[:])

            def epilogue(ps, ob, nt):
                osb = ostage.tile([P, 512], bf16, tag="osb")
                nc.scalar.activation(
                    osb[:], ps[:], ident,
                    bias=fb_sb[:, ob:ob + 1], scale=fs_sb[:, ob:ob + 1])
                nc.sync.dma_start(
                    out_o[ob * P:(ob + 1) * P, nt * 512:(nt + 1) * 512],
                    osb[:])

            # phase 2: finish the 4 held tiles first (banks already hot) while
            # the first reload wave pipelines in on the Act queue; then parked
            # tiles in small waves, reloads running ahead of epilogues.
            parked = tiles[:28]
            held_tiles = tiles[28:]
            sizes = [4, 4, 4, 4, 4, 4, 2, 1, 1]
            p2, pos = [], 0
            for sz in sizes:
                p2.append(list(enumerate(parked[pos:pos + sz], start=pos)))
                pos += sz

            def load_wave(wi):
                pss = [psum.tile([P, 512], f32, name="ps", tag="ps")
                       for _ in p2[wi]]
                for ps, (idx, _) in zip(pss, p2[wi]):
                    nc.scalar.copy(ps[:], parts[:, idx, :])
                return pss

            live = {0: load_wave(0)}
            for ib in range(KH, NB_I):
                for ps, (ob, nt) in zip(held, held_tiles):
                    mm(ps, ob, nt, ib, False, ib == NB_I - 1)
            for ps, (ob, nt) in zip(held, held_tiles):
                epilogue(ps, ob, nt)
            live[1] = load_wave(1)

            for wi, wave in enumerate(p2):
                pss = live.pop(wi)
                for ib in range(KH, NB_I):
                    for ps, (_, (ob, nt)) in zip(pss, wave):
                        mm(ps, ob, nt, ib, False, ib == NB_I - 1)
                for ps, (_, (ob, nt)) in zip(pss, wave):
                    epilogue(ps, ob, nt)
                if wi + 2 < len(p2):
                    live[wi + 2] = load_wave(wi + 2)

    nc.compile()
    _CACHE[key] = nc
    return nc


def _fast_consts(post_bin_scale, final_scale, final_bias, running_max):
    s = np.float32(running_max) / np.float32(QMAX)
    inv_se = np.float32(1.0) / (s + np.float32(EPS))
    sigma = np.float64(0.5) * (np.float64(s) + np.float64(EPS))
    pbs0 = np.float64(post_bin_scale.reshape(-1)[0])
    c0s = np.float32(sigma * pbs0)
    c1s = np.float32(np.float64(0.5) * sigma)
    fscol = np.ascontiguousarray(
        final_scale.astype(np.float32).reshape(NB_O, P).T)
    fbcol = np.ascontiguousarray(
        final_bias.astype(np.float32).reshape(NB_O, P).T)
    return inv_se, c0s, c1s, fscol, fbcol


def _fast_in_maps(x, weight, mask, fscol, fbcol):
    wT = np.ascontiguousarray(weight.T).reshape(NB_I, P, D_OUT)
    mT = np.ascontiguousarray(mask.T).view(np.uint8).reshape(NB_I, P, D_OUT)
    maps = []
    for c in range(N_CORES):
        xT = np.ascontiguousarray(x[c * N_SHARD:(c + 1) * N_SHARD].T)
        maps.append({
            "x": xT.reshape(NB_I, P, N_SHARD),
            "w": wT,
            "m": mT,
            "fs": fscol,
            "fb": fbcol,
        })
    return maps


# ---------------------------------------------------------------------------
# general fallback (non-uniform post_bin_scale): previous implementation
# ---------------------------------------------------------------------------

def _build_general(inv_se):
    key = ("nc", float(inv_se))
    if key in _CACHE:
        return _CACHE[key]

    import concourse.mybir as mybir
    import concourse.tile as tile
    from concourse import bacc

    xprep, wprep, _ = _register_ops()

    nc = bacc.Bacc(None, target_bir_lowering=False)
    bf16 = mybir.dt.bfloat16
    f32 = mybir.dt.float32

    x_in = nc.dram_tensor("x", [N_SHARD, D_IN], f32, kind="ExternalInput")
    w_in = nc.dram_tensor("w", [D_OUT, D_IN], f32, kind="ExternalInput")
    m_in = nc.dram_tensor("m", [D_OUT, D_IN], mybir.dt.uint8, kind="ExternalInput")
    c0_in = nc.dram_tensor("c0", [P, NB_O], f32, kind="ExternalInput")
    c1_in = nc.dram_tensor("c1", [P, NB_O], f32, kind="ExternalInput")
    fb_in = nc.dram_tensor("fb", [P, D_OUT], f32, kind="ExternalInput")
    out_o = nc.dram_tensor("out", [N_SHARD, D_OUT], f32, kind="ExternalOutput")

    from concourse.masks import make_identity

    with tile.TileContext(nc) as tc:
        with (
            tc.tile_pool(name="persist", bufs=1) as persist,
            tc.tile_pool(name="wlp", bufs=4) as wlp,
            tc.tile_pool(name="wpp", bufs=4) as wpp,
            tc.tile_pool(name="xlp", bufs=4) as xlp,
            tc.tile_pool(name="xbp", bufs=4) as xbp,
            tc.tile_pool(name="ostage", bufs=7) as ostage,
            tc.tile_pool(name="psum", bufs=6, space="PSUM") as psum,
            tc.tile_pool(name="tpsum", bufs=2, space="PSUM") as tpsum,
        ):
            wT = persist.tile([P, NB_O, NB_I, P], bf16, tag="wT")
            xqT = persist.tile([P, NJ, NB_I, P], bf16, tag="xqT")
            c0_sb = persist.tile([P, NB_O], f32, tag="c0")
            c1_sb = persist.tile([P, NB_O], f32, tag="c1")
            fb_sb = persist.tile([P, D_OUT], f32, tag="fb")
            ident = persist.tile([P, P], bf16, tag="ident")

            nc.sync.dma_start(fb_sb[:], fb_in[:])
            nc.sync.dma_start(c0_sb[:], c0_in[:])
            nc.sync.dma_start(c1_sb[:], c1_in[:])
            make_identity(nc, ident[:])

            def w_block(ob):
                wt = wlp.tile([P, D_IN], bf16, tag="w_bf16")
                mt = wlp.tile([P, D_IN], mybir.dt.uint8, tag="m_u8")
                nc.gpsimd.dma_start(wt[:], w_in[ob * P:(ob + 1) * P, :])
                nc.scalar.dma_start(mt[:], m_in[ob * P:(ob + 1) * P, :])
                w2 = wpp.tile([P, D_IN], bf16, tag="w2")
                nc.vector._custom_dve(
                    wprep, out=w2[:], in0=wt[:], in1=mt[:],
                    s0=c0_sb[:, ob:ob + 1], s1=c1_sb[:, ob:ob + 1], imm2=0.5)
                nc.sync.dma_start_transpose(wT[:, ob], w2[:])

            def x_block(j):
                xt = xlp.tile([P, D_IN], f32, tag="x_f32")
                nc.sync.dma_start(xt[:], x_in[j * P:(j + 1) * P, :])
                xb = xbp.tile([P, D_IN], bf16, tag="xb")
                nc.vector._custom_dve(
                    xprep, out=xb[:], in0=xt[:],
                    s0=float(inv_se), s1=MAGIC, imm2=QMAX)
                for b in range(NB_I):
                    tp = tpsum.tile([P, P], bf16, tag="xtp")
                    nc.tensor.transpose(tp[:], xb[:, b * P:(b + 1) * P], ident[:])
                    nc.scalar.copy(xqT[:, j, b, :], tp[:])

            for ob in range(OB_PER_T):
                w_block(ob)
            for j in range(NJ):
                x_block(j)
            for ob in range(OB_PER_T, NB_O):
                w_block(ob)

            for t in range(NT):
                for j in range(NJ):
                    ps = psum.tile([P, OT], f32, tag="ps")
                    for b in range(NB_I):
                        nc.tensor.matmul(
                            ps[:],
                            xqT[:, j, b, :],
                            wT[:, t * OB_PER_T:(t + 1) * OB_PER_T, b, :],
                            start=(b == 0), stop=(b == NB_I - 1))
                    osb = ostage.tile([P, OT], f32, tag="osb")
                    nc.vector.tensor_add(
                        osb[:], ps[:], fb_sb[:, t * OT:(t + 1) * OT])
                    nc.scalar.dma_start(
                        out_o[j * P:(j + 1) * P, t * OT:(t + 1) * OT], osb[:])

    nc.compile()
    _CACHE[key] = nc
    return nc


def _general_in_maps(x, weight, mask_u8, c0, c1, fb):
    maps = []
    for c in range(N_CORES):
        maps.append({
            "x": np.ascontiguousarray(x[c * N_SHARD:(c + 1) * N_SHARD]),
            "w": weight,
            "m": mask_u8,
            "c0": c0,
            "c1": c1,
            "fb": fb,
        })
    return maps


def _general_consts(post_bin_scale, final_scale, final_bias, running_max):
    s = np.float32(running_max) / np.float32(QMAX)
    inv_se = np.float32(1.0) / (s + np.float32(EPS))
    sigma = np.float64(0.5) * (np.float64(s) + np.float64(EPS))
    c0_all = (sigma * final_scale.astype(np.float64)
              * post_bin_scale.reshape(-1).astype(np.float64)).astype(np.float32)
    c1_all = (np.float64(0.5) * sigma
              * final_scale.astype(np.float64)).astype(np.float32)
    c0 = np.ascontiguousarray(c0_all.reshape(NB_O, P).T)
    c1 = np.ascontiguousarray(c1_all.reshape(NB_O, P).T)
    fb = np.ascontiguousarray(
        np.broadcast_to(final_bias.astype(np.float32), (P, D_OUT)))
    return inv_se, c0, c1, fb


def _run_spmd(nc, maps):
    """Execute with retry: axon-tunneled devices can transiently fail."""
    from concourse.bass_utils import run_bass_kernel_spmd
    for attempt in range(3):
        try:
            return run_bass_kernel_spmd(nc, maps, core_ids=list(range(N_CORES)))
        except Exception:  # noqa: BLE001 - retrying device-side faults
            if attempt == 2:
                raise
            import gc
            import time as _time
            gc.collect()
            try:
                import jax
                jax.clear_caches()
                import jax.extend as _jex
                _jex.backend.clear_backends()
            except Exception:
                pass
            _time.sleep(10)


def prepare(x, weight, post_bin_scale, final_scale, final_bias, running_max,
            sprinkle_mask):
    """Build (compile) the module and the per-core input maps."""
    x = np.asarray(x, dtype=np.float32)
    weight = np.ascontiguousarray(np.asarray(weight, dtype=np.float32))
    mask = np.asarray(sprinkle_mask)
    pbs = np.asarray(post_bin_scale, dtype=np.float32)
    fs = np.asarray(final_scale, dtype=np.float32)
    fb = np.asarray(final_bias, dtype=np.float32)
    rm = float(np.asarray(running_max))

    if np.all(pbs.reshape(-1) == pbs.reshape(-1)[0]):
        inv_se, c0s, c1s, fscol, fbcol = _fast_consts(pbs, fs, fb, rm)
        nc = _build_fast(inv_se, c0s, c1s)
        maps = _fast_in_maps(x, weight, mask, fscol, fbcol)
        fast = True
    else:
        inv_se, c0, c1, fbb = _general_consts(pbs, fs, fb, rm)
        nc = _build_general(inv_se)
        maps = _general_in_maps(
            x, weight, np.ascontiguousarray(mask).view(np.uint8), c0, c1, fbb)
        fast = False
    return nc, maps, fast


def kernel(x, weight, post_bin_scale, final_scale, final_bias, running_max,
           sprinkle_mask):
    nc, maps, fast = prepare(x, weight, post_bin_scale, final_scale,
                             final_bias, running_max, sprinkle_mask)
    res = _run_spmd(nc, maps)
    if fast:
        out = np.concatenate(
            [np.asarray(res.results[c]["out"]).astype(np.float32).T
             for c in range(N_CORES)], axis=0)
    else:
        out = np.concatenate(
            [res.results[c]["out"] for c in range(N_CORES)], axis=0)
    return np.ascontiguousarray(out)


# revision 28
# speedup vs baseline: 1.4673x; 1.0085x over previous
"""BitLinear (quantized-activation, binarized-sprinkled-weight linear) Trainium2 kernel.

Data-parallel over the token dim N across 8 NeuronCores.

Fast path (post_bin_scale uniform, which holds for this problem's inputs):
  * all operands are laid out HOST-side (pure transposes/views) so the
    contraction dim i lands on SBUF partitions with no device transposes:
       xT [IN, N_SHARD] f32 (per core),  wT [IN, OUT] f32,  mT [IN, OUT] u8
  * w/x are DMA-cast to bf16 on load (gpsimd SWDGE); one fused DVE op per
    operand block:
       xb = t + clip(round(t), +-127),  t = x*inv_se          (bf16)
       W2 = m ? sign(w)*C0 : w*C1 + sign(w)*C0*0.5            (bf16)
    with C0 = sigma*pbs, C1 = 0.5*sigma global scalars (sigma = 0.5*(s+eps)),
    so that  out[n,o] = (xb @ W2^T)*fs[o] + fb[o]
  * prep is emitted in dependency order at half-block granularity so DVE
    production (the prep bottleneck, custom ops run 1x) matches the PE's
    PSUM-capacity-limited consumption exactly -- no matmul pacing stalls
  * matmuls produce PSUM tiles [o=128, n=512] (o on partitions), 16 k-steps.
    PSUM only holds 8 such tiles, so the contraction is split 8+8: during
    prep, every output tile's first half accumulates in waves of 8 and is
    parked in SBUF as bf16 partials; after prep the halves are reloaded
    (Act engine, software-pipelined two waves deep) and finished
  * epilogue on the Activation engine: Identity with per-partition scale=fs,
    bias=fb, writing bf16; stores go out transposed [OUT, N_SHARD] and the
    host transposes back / upcasts (layout-only work).

General path (non-uniform post_bin_scale): previous per-partition-constant
implementation, kept verbatim as a fallback.
"""

import numpy as np

N_CORES = 8
N_TOK, D_IN, D_OUT = 8192, 2048, 2048
N_SHARD = N_TOK // N_CORES          # 1024 rows of x per core
P = 128
NJ = N_SHARD // P                   # 8 n-blocks per core
NB_I = D_IN // P                    # 16 i-blocks (contraction)
NB_O = D_OUT // P                   # 16 o-blocks
OT = 512                            # o-tile (one PSUM bank)
NT = D_OUT // OT                    # 4 o-tiles
OB_PER_T = OT // P                  # 4 o-blocks per o-tile

MM_NT = N_SHARD // 512              # fast path: 2 moving-dim tiles of 512

QMAX = 127.0
EPS = 1e-6
MAGIC = 12582912.0                  # 1.5 * 2**23: fp32 RNE round-to-int trick

_CACHE = {}


def _register_ops():
    """Define the two fused DVE ops (idempotent)."""
    from concourse import dve_ops
    from concourse.dve_spec import (
        Spec, Src0, Src1, C0, C1, C2, Zero, select, minn, maxx, lower, _has_src1,
    )
    from concourse.dve_uop import DveOpSpec

    def register(name, spec):
        for op in dve_ops.OPS:
            if op.name == name:
                return op
        ver = "v3"
        tmp = DveOpSpec(name=name, opcode=0, uops=lower(spec, ver=ver),
                        rd1_en=_has_src1(spec))
        op = dve_ops.DveOp(name, spec, subdim=False,
                           uops_sha={ver: tmp.sha(ver)})
        dve_ops.OPS.append(op)
        dve_ops._SUB_OPCODE_FOR_NAME[name] = (
            max(dve_ops._SUB_OPCODE_FOR_NAME.values()) + 1)
        dve_ops.CUSTOM_DVE_SPECS[name] = spec
        return op

    # out = t + clip(round(t), +-imm2), t = x*s0   (s1 = MAGIC)
    _t = Src0 * C0
    _r = (_t + C1) - C1
    _rc = minn(maxx(_r, Zero - C2), C2)
    xprep = register("XPREP_BITLIN", Spec(
        body=_t + _rc,
        reference=lambda in0, in1, s0, s1, imm2: (
            (lambda t: t + np.clip(np.round(t), -imm2, imm2))(
                in0.astype(np.float32) * s0)),
    ))

    # h = select(w>=0, s0, -s0); out = select(m>0, h, w*s1 + h*imm2)
    _h = select(Src0 >= Zero, C0, Zero - C0)
    wprep = register("WPREP_BITLIN", Spec(
        body=select(Src1 > Zero, _h, Src0 * C1 + _h * C2),
        reference=lambda in0, in1, s0, s1, imm2: (
            (lambda h: np.where(in1 > 0, h,
                                in0.astype(np.float32) * s1 + h * imm2))(
                np.where(in0 >= 0, s0, -s0))),
    ))

    # two-constant variant (no imm2 -> allows 2-free-dim src1):
    # hh = select(w>=0, s0, -s0); out = select(m>0, hh+hh, w*s1 + hh)
    # with s0 = 0.5*C0 so that hh+hh = sign(w)*C0.
    _hh = select(Src0 >= Zero, C0, Zero - C0)
    wprep2 = register("WPREP2_BITLIN", Spec(
        body=select(Src1 > Zero, _hh + _hh, Src0 * C1 + _hh),
        reference=lambda in0, in1, s0, s1, imm2: (
            (lambda h: np.where(in1 > 0, h + h,
                                in0.astype(np.float32) * s1 + h))(
                np.where(in0 >= 0, s0, -s0))),
    ))
    return xprep, wprep, wprep2


def _build_fast(inv_se, c0s, c1s):
    """Fast-path per-core Bass module (uniform post_bin_scale).

    inv_se, c0s, c1s are global scalars baked into the DVE ops.
    """
    key = ("fast", float(inv_se), float(c0s), float(c1s))
    if key in _CACHE:
        return _CACHE[key]

    import concourse.mybir as mybir
    import concourse.tile as tile
    from concourse import bacc

    xprep, _, wprep2 = _register_ops()

    nc = bacc.Bacc(None, target_bir_lowering=False)
    bf16 = mybir.dt.bfloat16
    f32 = mybir.dt.float32
    ident = mybir.ActivationFunctionType.Identity

    x_in = nc.dram_tensor("x", [NB_I, P, N_SHARD], f32, kind="ExternalInput")
    w_in = nc.dram_tensor("w", [NB_I, P, D_OUT], f32, kind="ExternalInput")
    m_in = nc.dram_tensor("m", [NB_I, P, D_OUT], mybir.dt.uint8,
                          kind="ExternalInput")
    fs_in = nc.dram_tensor("fs", [P, NB_O], f32, kind="ExternalInput")
    fb_in = nc.dram_tensor("fb", [P, NB_O], f32, kind="ExternalInput")
    out_o = nc.dram_tensor("out", [D_OUT, N_SHARD], bf16, kind="ExternalOutput")

    KH = NB_I // 2                  # contraction split: 8 + 8 k-steps
    # wave order matches operand production order: (nt0,ob0-7) needs only the
    # Xa/Wa halves, (nt1,ob0-7) adds Xb, (nt0,ob8-15) adds Wb, (nt1,ob8-15)
    # needs everything -- so no wave ever waits on prep at a phase boundary
    tiles = ([(ob, 0) for ob in range(8)] + [(ob, 1) for ob in range(8)]
             + [(ob, 0) for ob in range(8, 16)]
             + [(ob, 1) for ob in range(8, 16)])
    waves = [tiles[w0:w0 + 8] for w0 in range(0, len(tiles), 8)]

    with tile.TileContext(nc) as tc:
        with (
            tc.tile_pool(name="persist", bufs=1) as persist,
            tc.tile_pool(name="wlp", bufs=6) as wlp,
            tc.tile_pool(name="mlp", bufs=6) as mlp,
            tc.tile_pool(name="xlp", bufs=5) as xlp,
            tc.tile_pool(name="ostage", bufs=8) as ostage,
            tc.tile_pool(name="psum", bufs=8, space="PSUM") as psum,
        ):
            w2 = persist.tile([P, NB_I, D_OUT], bf16, tag="w2")   # [i_in, ib, o]
            xb = persist.tile([P, NB_I, N_SHARD], bf16, tag="xb")  # [i_in, ib, n]
            # bf16 partial sums for the first contraction half (PSUM is only
            # 8 banks; parking partials in SBUF lets all 32 output tiles
            # overlap the operand-prep phase)
            parts = persist.tile([P, len(tiles), 512], bf16, tag="parts")
            fs_sb = persist.tile([P, NB_O], f32, tag="fs")
            fb_sb = persist.tile([P, NB_O], f32, tag="fb")

            # PE warm-up source: a zeroed bf16 tile for dummy matmuls that
            # keep the tensor engine busy through the prep-latency window so
            # real matmuls start at full p-state.
            warm = persist.tile([P, 512], bf16, tag="warm")
            nc.vector.memset(warm[:], 0.0)

            # Operand prep in dependency order. Phase-1 wave 1 (nt=0, ob 0-7)
            # consumes one (Xa, Wa) half-block pair per 1.7 us; emitting
            # exactly those halves first makes DVE production match PE
            # consumption, eliminating wave-1 pacing stalls. Remaining halves
            # follow in the order later waves need them.
            XHF, WHF = N_SHARD // 2, D_OUT // 2

            def x_op(ib, lo, sz, hwdge=False):
                # hwdge=True: plain f32 load on the sync HWDGE ring. Used in
                # the pacing-critical first phase so the Pool engine's SWDGE
                # desc-gen (994ns fixed per DMA) only carries the w loads and
                # operand production keeps up with wave-1 consumption.
                if hwdge:
                    xt = xlp.tile([P, sz], f32, name="xt", tag="x_f32")
                    nc.sync.dma_start(xt[:],
                                      x_in[ib, :, lo:lo + sz])
                else:
                    xt = xlp.tile([P, sz], bf16, name="xt", tag="x_bf16")
                    nc.gpsimd.dma_start(xt[:],
                                        x_in[ib, :, lo:lo + sz])
                nc.vector._custom_dve(
                    xprep, out=xb[:, ib, lo:lo + sz], in0=xt[:],
                    s0=float(inv_se), s1=MAGIC, imm2=QMAX)

            def w_op(ib, lo, sz):
                mt = mlp.tile([P, sz], mybir.dt.uint8, name="mt", tag="m_u8")
                nc.sync.dma_start(mt[:], m_in[ib, :, lo:lo + sz])
                wt = wlp.tile([P, sz], bf16, name="wt", tag="w_bf16")
                nc.gpsimd.dma_start(wt[:], w_in[ib, :, lo:lo + sz])
                nc.vector._custom_dve(
                    wprep2, out=w2[:, ib, lo:lo + sz], in0=wt[:], in1=mt[:],
                    s0=float(0.5 * c0s), s1=float(c1s))

            # Wave 1 deps, paced 1:1. Block 0 takes minimum-latency load
            # paths: its x half rides the sync HWDGE ring as plain f32 (no
            # Pool desc-gen), making the first w load the Pool generator's
            # FIRST job -- the first matmul starts ~1us sooner. Blocks 1-7's
            # x halves load two per SWDGE DMA (one desc-gen each),
            # prefetched ahead of use.
            xt0 = xlp.tile([P, XHF], f32, name="xt0", tag="x0_f32")
            nc.sync.dma_start(xt0[:], x_in[0, :, 0:XHF])
            mt0 = mlp.tile([P, WHF], mybir.dt.uint8, name="mt0", tag="m_u8")
            nc.sync.dma_start(mt0[:], m_in[0, :, 0:WHF])
            wt0 = wlp.tile([P, WHF], bf16, name="wt0", tag="w_bf16")
            nc.gpsimd.dma_start(wt0[:], w_in[0, :, 0:WHF])

            xa_src = {}

            def xa_load(ib0, nb):
                t2 = xlp.tile([P, nb, XHF], bf16, name="t2", tag="x_bf16p")
                nc.gpsimd.dma_start(
                    t2[:], x_in[ib0:ib0 + nb, :, 0:XHF].transpose([1, 0, 2]))
                for j in range(nb):
                    xa_src[ib0 + j] = t2[:, j, :]

            xa_load(1, 2)
            nc.vector._custom_dve(
                xprep, out=xb[:, 0, 0:XHF], in0=xt0[:],
                s0=float(inv_se), s1=MAGIC, imm2=QMAX)
            nc.vector._custom_dve(
                wprep2, out=w2[:, 0, 0:WHF], in0=wt0[:], in1=mt0[:],
                s0=float(0.5 * c0s), s1=float(c1s))
            batches = [(3, 2), (5, 2), (7, 1)]
            bi = 0
            for ib in range(1, KH):
                nc.vector._custom_dve(
                    xprep, out=xb[:, ib, 0:XHF], in0=xa_src[ib],
                    s0=float(inv_se), s1=MAGIC, imm2=QMAX)
                w_op(ib, 0, WHF)
                if ib % 2 == 0 and bi < len(batches):
                    xa_load(*batches[bi])
                    bi += 1
            for ib in range(KH):                 # wave 2 deps (nt1, ob 0-7)
                x_op(ib, XHF, XHF)
            for ib in range(KH):                 # wave 3 deps (nt0, ob 8-15)
                w_op(ib, WHF, WHF)
            for ib in range(KH, NB_I):           # second contraction half
                x_op(ib, 0, N_SHARD)
                w_op(ib, 0, D_OUT)

            # epilogue constants (needed only after the first full tiles
            # finish, so loaded after the prep stream is underway)
            nc.scalar.dma_start(fs_sb[:], fs_in[:])
            nc.scalar.dma_start(fb_sb[:], fb_in[:])

            def mm(ps, ob, nt, ib, start, stop):
                nc.tensor.matmul(
                    ps[:],
                    w2[:, ib, ob * P:(ob + 1) * P],
                    xb[:, ib, nt * 512:(nt + 1) * 512],
                    start=start, stop=stop, skip_group_check=True)

            # phase 1 (overlaps prep): first contraction half for every tile,
            # k-outer within each 8-tile wave; partials parked in SBUF bf16.
            # The last wave's final 4 tiles keep their banks: phase 2
            # continues them in place while the first reloads pipeline in.
            held = []
            for wi, wave in enumerate(waves):
                pss = [psum.tile([P, 512], f32, name="ps", tag="ps")
                       for _ in wave]
                if wi == 0:
                    # dummies into pss[0]; the real ib-0 matmul resets it
                    # (start=True), so only timing is affected.
                    for _ in range(7):
                        nc.tensor.matmul(pss[0][:], warm[:, 0:P], warm[:],
                                         start=True, stop=True,
                                         skip_group_check=True)
                for ib in range(KH):
                    for ps, (ob, nt) in zip(pss, wave):
                        mm(ps, ob, nt, ib, ib == 0, ib == KH - 1)
                if wi == len(waves) - 1:
                    for k in range(4):
                        nc.scalar.copy(parts[:, wi * 8 + k, :], pss[k][:])
                    held = pss[4:]
                else:
                    for k, ps in enumerate(pss):
                        nc.scalar.copy(parts[:, wi * 8 + k, :], ps[:])

            def epilogue(ps, ob, nt):
                osb = ostage.tile([P, 512], bf16, tag="osb")
                nc.scalar.activation(
                    osb[:], ps[:], ident,
                    bias=fb_sb[:, ob:ob + 1], scale=fs_sb[:, ob:ob + 1])
                nc.sync.dma_start(
                    out_o[ob * P:(ob + 1) * P, nt * 512:(nt + 1) * 512],
                    osb[:])

            # phase 2: finish the 4 held tiles first (banks already hot) while
            # the first reload wave pipelines in on the Act queue; then parked
            # tiles in small waves, reloads running ahead of epilogues.
            parked = tiles[:28]
            held_tiles = tiles[28:]
            sizes = [4, 4, 4, 4, 4, 4, 2, 1, 1]
            p2, pos = [], 0
            for sz in sizes:
                p2.append(list(enumerate(parked[pos:pos + sz], start=pos)))
                pos += sz

            live_half = []

            def load_wave(wi):
                if wi == len(p2) - 1 and len(p2[wi]) == 1:
                    idx = p2[wi][0][0]
                    psA = psum.tile([P, 256], f32, name="psA", tag="ps")
                    psB = psum.tile([P, 256], f32, name="psB", tag="ps")
                    nc.scalar.copy(psA[:], parts[:, idx, 0:256])
                    nc.scalar.copy(psB[:], parts[:, idx, 256:512])
                    live_half.append(psB)
                    return [psA]
                pss = [psum.tile([P, 512], f32, name="ps", tag="ps")
                       for _ in p2[wi]]
                for ps, (idx, _) in zip(pss, p2[wi]):
                    nc.scalar.copy(ps[:], parts[:, idx, :])
                return pss

            live = {0: load_wave(0)}
            for ib in range(KH, NB_I):
                for ps, (ob, nt) in zip(held, held_tiles):
                    mm(ps, ob, nt, ib, False, ib == NB_I - 1)
            for ps, (ob, nt) in zip(held, held_tiles):
                epilogue(ps, ob, nt)
            live[1] = load_wave(1)

            for wi, wave in enumerate(p2):
                pss = live.pop(wi)
                if wi == len(p2) - 1 and len(wave) == 1:
                    # final tile in two independent 256-col halves: half A's
                    # epilogue+store launch while half B's matmuls run, and
                    # the closing drain chain shrinks to 256-wide ops
                    idx, (ob, nt) = wave[0]
                    psA, psB = pss[0], live_half[0]
                    for half, ps in ((0, psA), (1, psB)):
                        lo = nt * 512 + half * 256
                        for ib in range(KH, NB_I):
                            nc.tensor.matmul(
                                ps[:], w2[:, ib, ob * P:(ob + 1) * P],
                                xb[:, ib, lo:lo + 256],
                                start=False, stop=(ib == NB_I - 1),
                                skip_group_check=True)
                        osb = ostage.tile([P, 256], bf16, name="osbh",
                                          tag="osbh")
                        nc.scalar.activation(
                            osb[:], ps[:], ident,
                            bias=fb_sb[:, ob:ob + 1], scale=fs_sb[:, ob:ob + 1])
                        nc.sync.dma_start(
                            out_o[ob * P:(ob + 1) * P, lo:lo + 256], osb[:])
                    continue
                for ib in range(KH, NB_I):
                    for ps, (_, (ob, nt)) in zip(pss, wave):
                        mm(ps, ob, nt, ib, False, ib == NB_I - 1)
                for ps, (_, (ob, nt)) in zip(pss, wave):
                    epilogue(ps, ob, nt)
                if wi + 2 < len(p2):
                    live[wi + 2] = load_wave(wi + 2)

    nc.compile()
    _CACHE[key] = nc
    return nc


def _fast_consts(post_bin_scale, final_scale, final_bias, running_max):
    s = np.float32(running_max) / np.float32(QMAX)
    inv_se = np.float32(1.0) / (s + np.float32(EPS))
    sigma = np.float64(0.5) * (np.float64(s) + np.float64(EPS))
    pbs0 = np.float64(post_bin_scale.reshape(-1)[0])
    c0s = np.float32(sigma * pbs0)
    c1s = np.float32(np.float64(0.5) * sigma)
    fscol = np.ascontiguousarray(
        final_scale.astype(np.float32).reshape(NB_O, P).T)
    fbcol = np.ascontiguousarray(
        final_bias.astype(np.float32).reshape(NB_O, P).T)
    return inv_se, c0s, c1s, fscol, fbcol


def _fast_in_maps(x, weight, mask, fscol, fbcol):
    wT = np.ascontiguousarray(weight.T).reshape(NB_I, P, D_OUT)
    mT = np.ascontiguousarray(mask.T).view(np.uint8).reshape(NB_I, P, D_OUT)
    maps = []
    for c in range(N_CORES):
        xT = np.ascontiguousarray(x[c * N_SHARD:(c + 1) * N_SHARD].T)
        maps.append({
            "x": xT.reshape(NB_I, P, N_SHARD),
            "w": wT,
            "m": mT,
            "fs": fscol,
            "fb": fbcol,
        })
    return maps


# ---------------------------------------------------------------------------
# general fallback (non-uniform post_bin_scale): previous implementation
# ---------------------------------------------------------------------------

def _build_general(inv_se):
    key = ("nc", float(inv_se))
    if key in _CACHE:
        return _CACHE[key]

    import concourse.mybir as mybir
    import concourse.tile as tile
    from concourse import bacc

    xprep, wprep, _ = _register_ops()

    nc = bacc.Bacc(None, target_bir_lowering=False)
    bf16 = mybir.dt.bfloat16
    f32 = mybir.dt.float32

    x_in = nc.dram_tensor("x", [N_SHARD, D_IN], f32, kind="ExternalInput")
    w_in = nc.dram_tensor("w", [D_OUT, D_IN], f32, kind="ExternalInput")
    m_in = nc.dram_tensor("m", [D_OUT, D_IN], mybir.dt.uint8, kind="ExternalInput")
    c0_in = nc.dram_tensor("c0", [P, NB_O], f32, kind="ExternalInput")
    c1_in = nc.dram_tensor("c1", [P, NB_O], f32, kind="ExternalInput")
    fb_in = nc.dram_tensor("fb", [P, D_OUT], f32, kind="ExternalInput")
    out_o = nc.dram_tensor("out", [N_SHARD, D_OUT], f32, kind="ExternalOutput")

    from concourse.masks import make_identity

    with tile.TileContext(nc) as tc:
        with (
            tc.tile_pool(name="persist", bufs=1) as persist,
            tc.tile_pool(name="wlp", bufs=4) as wlp,
            tc.tile_pool(name="wpp", bufs=4) as wpp,
            tc.tile_pool(name="xlp", bufs=4) as xlp,
            tc.tile_pool(name="xbp", bufs=4) as xbp,
            tc.tile_pool(name="ostage", bufs=7) as ostage,
            tc.tile_pool(name="psum", bufs=6, space="PSUM") as psum,
            tc.tile_pool(name="tpsum", bufs=2, space="PSUM") as tpsum,
        ):
            wT = persist.tile([P, NB_O, NB_I, P], bf16, tag="wT")
            xqT = persist.tile([P, NJ, NB_I, P], bf16, tag="xqT")
            c0_sb = persist.tile([P, NB_O], f32, tag="c0")
            c1_sb = persist.tile([P, NB_O], f32, tag="c1")
            fb_sb = persist.tile([P, D_OUT], f32, tag="fb")
            ident = persist.tile([P, P], bf16, tag="ident")

            nc.sync.dma_start(fb_sb[:], fb_in[:])
            nc.sync.dma_start(c0_sb[:], c0_in[:])
            nc.sync.dma_start(c1_sb[:], c1_in[:])
            make_identity(nc, ident[:])

            def w_block(ob):
                wt = wlp.tile([P, D_IN], bf16, tag="w_bf16")
                mt = wlp.tile([P, D_IN], mybir.dt.uint8, tag="m_u8")
                nc.gpsimd.dma_start(wt[:], w_in[ob * P:(ob + 1) * P, :])
                nc.scalar.dma_start(mt[:], m_in[ob * P:(ob + 1) * P, :])
                w2 = wpp.tile([P, D_IN], bf16, tag="w2")
                nc.vector._custom_dve(
                    wprep, out=w2[:], in0=wt[:], in1=mt[:],
                    s0=c0_sb[:, ob:ob + 1], s1=c1_sb[:, ob:ob + 1], imm2=0.5)
                nc.sync.dma_start_transpose(wT[:, ob], w2[:])

            def x_block(j):
                xt = xlp.tile([P, D_IN], f32, tag="x_f32")
                nc.sync.dma_start(xt[:], x_in[j * P:(j + 1) * P, :])
                xb = xbp.tile([P, D_IN], bf16, tag="xb")
                nc.vector._custom_dve(
                    xprep, out=xb[:], in0=xt[:],
                    s0=float(inv_se), s1=MAGIC, imm2=QMAX)
                for b in range(NB_I):
                    tp = tpsum.tile([P, P], bf16, tag="xtp")
                    nc.tensor.transpose(tp[:], xb[:, b * P:(b + 1) * P], ident[:])
                    nc.scalar.copy(xqT[:, j, b, :], tp[:])

            for ob in range(OB_PER_T):
                w_block(ob)
            for j in range(NJ):
                x_block(j)
            for ob in range(OB_PER_T, NB_O):
                w_block(ob)

            for t in range(NT):
                for j in range(NJ):
                    ps = psum.tile([P, OT], f32, tag="ps")
                    for b in range(NB_I):
                        nc.tensor.matmul(
                            ps[:],
                            xqT[:, j, b, :],
                            wT[:, t * OB_PER_T:(t + 1) * OB_PER_T, b, :],
                            start=(b == 0), stop=(b == NB_I - 1))
                    osb = ostage.tile([P, OT], f32, tag="osb")
                    nc.vector.tensor_add(
                        osb[:], ps[:], fb_sb[:, t * OT:(t + 1) * OT])
                    nc.scalar.dma_start(
                        out_o[j * P:(j + 1) * P, t * OT:(t + 1) * OT], osb[:])

    nc.compile()
    _CACHE[key] = nc
    return nc


def _general_in_maps(x, weight, mask_u8, c0, c1, fb):
    maps = []
    for c in range(N_CORES):
        maps.append({
            "x": np.ascontiguousarray(x[c * N_SHARD:(c + 1) * N_SHARD]),
            "w": weight,
            "m": mask_u8,
            "c0": c0,
            "c1": c1,
            "fb": fb,
        })
    return maps


def _general_consts(post_bin_scale, final_scale, final_bias, running_max):
    s = np.float32(running_max) / np.float32(QMAX)
    inv_se = np.float32(1.0) / (s + np.float32(EPS))
    sigma = np.float64(0.5) * (np.float64(s) + np.float64(EPS))
    c0_all = (sigma * final_scale.astype(np.float64)
              * post_bin_scale.reshape(-1).astype(np.float64)).astype(np.float32)
    c1_all = (np.float64(0.5) * sigma
              * final_scale.astype(np.float64)).astype(np.float32)
    c0 = np.ascontiguousarray(c0_all.reshape(NB_O, P).T)
    c1 = np.ascontiguousarray(c1_all.reshape(NB_O, P).T)
    fb = np.ascontiguousarray(
        np.broadcast_to(final_bias.astype(np.float32), (P, D_OUT)))
    return inv_se, c0, c1, fb


def _run_spmd(nc, maps):
    """Execute with retry: axon-tunneled devices can transiently fail."""
    from concourse.bass_utils import run_bass_kernel_spmd
    for attempt in range(3):
        try:
            return run_bass_kernel_spmd(nc, maps, core_ids=list(range(N_CORES)))
        except Exception:  # noqa: BLE001 - retrying device-side faults
            if attempt == 2:
                raise
            import gc
            import time as _time
            gc.collect()
            try:
                import jax
                jax.clear_caches()
                import jax.extend as _jex
                _jex.backend.clear_backends()
            except Exception:
                pass
            _time.sleep(10)


def prepare(x, weight, post_bin_scale, final_scale, final_bias, running_max,
            sprinkle_mask):
    """Build (compile) the module and the per-core input maps."""
    x = np.asarray(x, dtype=np.float32)
    weight = np.ascontiguousarray(np.asarray(weight, dtype=np.float32))
    mask = np.asarray(sprinkle_mask)
    pbs = np.asarray(post_bin_scale, dtype=np.float32)
    fs = np.asarray(final_scale, dtype=np.float32)
    fb = np.asarray(final_bias, dtype=np.float32)
    rm = float(np.asarray(running_max))

    if np.all(pbs.reshape(-1) == pbs.reshape(-1)[0]):
        inv_se, c0s, c1s, fscol, fbcol = _fast_consts(pbs, fs, fb, rm)
        nc = _build_fast(inv_se, c0s, c1s)
        maps = _fast_in_maps(x, weight, mask, fscol, fbcol)
        fast = True
    else:
        inv_se, c0, c1, fbb = _general_consts(pbs, fs, fb, rm)
        nc = _build_general(inv_se)
        maps = _general_in_maps(
            x, weight, np.ascontiguousarray(mask).view(np.uint8), c0, c1, fbb)
        fast = False
    return nc, maps, fast


def kernel(x, weight, post_bin_scale, final_scale, final_bias, running_max,
           sprinkle_mask):
    nc, maps, fast = prepare(x, weight, post_bin_scale, final_scale,
                             final_bias, running_max, sprinkle_mask)
    res = _run_spmd(nc, maps)
    if fast:
        out = np.concatenate(
            [np.asarray(res.results[c]["out"]).astype(np.float32).T
             for c in range(N_CORES)], axis=0)
    else:
        out = np.concatenate(
            [res.results[c]["out"] for c in range(N_CORES)], axis=0)
    return np.ascontiguousarray(out)


# revision 29
# speedup vs baseline: 1.4679x; 1.0005x over previous
"""BitLinear (quantized-activation, binarized-sprinkled-weight linear) Trainium2 kernel.

Data-parallel over the token dim N across 8 NeuronCores.

Fast path (post_bin_scale uniform, which holds for this problem's inputs):
  * all operands are laid out HOST-side (pure transposes/views) so the
    contraction dim i lands on SBUF partitions with no device transposes:
       xT [IN, N_SHARD] f32 (per core),  wT [IN, OUT] f32,  mT [IN, OUT] u8
  * w/x are DMA-cast to bf16 on load (gpsimd SWDGE); one fused DVE op per
    operand block:
       xb = t + clip(round(t), +-127),  t = x*inv_se          (bf16)
       W2 = m ? sign(w)*C0 : w*C1 + sign(w)*C0*0.5            (bf16)
    with C0 = sigma*pbs, C1 = 0.5*sigma global scalars (sigma = 0.5*(s+eps)),
    so that  out[n,o] = (xb @ W2^T)*fs[o] + fb[o]
  * prep is emitted in dependency order at half-block granularity so DVE
    production (the prep bottleneck, custom ops run 1x) matches the PE's
    PSUM-capacity-limited consumption exactly -- no matmul pacing stalls
  * matmuls produce PSUM tiles [o=128, n=512] (o on partitions), 16 k-steps.
    PSUM only holds 8 such tiles, so the contraction is split 8+8: during
    prep, every output tile's first half accumulates in waves of 8 and is
    parked in SBUF as bf16 partials; after prep the halves are reloaded
    (Act engine, software-pipelined two waves deep) and finished
  * epilogue on the Activation engine: Identity with per-partition scale=fs,
    bias=fb, writing bf16; stores go out transposed [OUT, N_SHARD] and the
    host transposes back / upcasts (layout-only work).

General path (non-uniform post_bin_scale): previous per-partition-constant
implementation, kept verbatim as a fallback.
"""

import numpy as np

N_CORES = 8
N_TOK, D_IN, D_OUT = 8192, 2048, 2048
N_SHARD = N_TOK // N_CORES          # 1024 rows of x per core
P = 128
NJ = N_SHARD // P                   # 8 n-blocks per core
NB_I = D_IN // P                    # 16 i-blocks (contraction)
NB_O = D_OUT // P                   # 16 o-blocks
OT = 512                            # o-tile (one PSUM bank)
NT = D_OUT // OT                    # 4 o-tiles
OB_PER_T = OT // P                  # 4 o-blocks per o-tile

MM_NT = N_SHARD // 512              # fast path: 2 moving-dim tiles of 512

QMAX = 127.0
EPS = 1e-6
MAGIC = 12582912.0                  # 1.5 * 2**23: fp32 RNE round-to-int trick

_CACHE = {}


def _register_ops():
    """Define the two fused DVE ops (idempotent)."""
    from concourse import dve_ops
    from concourse.dve_spec import (
        Spec, Src0, Src1, C0, C1, C2, Zero, select, minn, maxx, lower, _has_src1,
    )
    from concourse.dve_uop import DveOpSpec

    def register(name, spec):
        for op in dve_ops.OPS:
            if op.name == name:
                return op
        ver = "v3"
        tmp = DveOpSpec(name=name, opcode=0, uops=lower(spec, ver=ver),
                        rd1_en=_has_src1(spec))
        op = dve_ops.DveOp(name, spec, subdim=False,
                           uops_sha={ver: tmp.sha(ver)})
        dve_ops.OPS.append(op)
        dve_ops._SUB_OPCODE_FOR_NAME[name] = (
            max(dve_ops._SUB_OPCODE_FOR_NAME.values()) + 1)
        dve_ops.CUSTOM_DVE_SPECS[name] = spec
        return op

    # out = t + clip(round(t), +-imm2), t = x*s0   (s1 = MAGIC)
    _t = Src0 * C0
    _r = (_t + C1) - C1
    _rc = minn(maxx(_r, Zero - C2), C2)
    xprep = register("XPREP_BITLIN", Spec(
        body=_t + _rc,
        reference=lambda in0, in1, s0, s1, imm2: (
            (lambda t: t + np.clip(np.round(t), -imm2, imm2))(
                in0.astype(np.float32) * s0)),
    ))

    # h = select(w>=0, s0, -s0); out = select(m>0, h, w*s1 + h*imm2)
    _h = select(Src0 >= Zero, C0, Zero - C0)
    wprep = register("WPREP_BITLIN", Spec(
        body=select(Src1 > Zero, _h, Src0 * C1 + _h * C2),
        reference=lambda in0, in1, s0, s1, imm2: (
            (lambda h: np.where(in1 > 0, h,
                                in0.astype(np.float32) * s1 + h * imm2))(
                np.where(in0 >= 0, s0, -s0))),
    ))

    # two-constant variant (no imm2 -> allows 2-free-dim src1):
    # hh = select(w>=0, s0, -s0); out = select(m>0, hh+hh, w*s1 + hh)
    # with s0 = 0.5*C0 so that hh+hh = sign(w)*C0.
    _hh = select(Src0 >= Zero, C0, Zero - C0)
    wprep2 = register("WPREP2_BITLIN", Spec(
        body=select(Src1 > Zero, _hh + _hh, Src0 * C1 + _hh),
        reference=lambda in0, in1, s0, s1, imm2: (
            (lambda h: np.where(in1 > 0, h + h,
                                in0.astype(np.float32) * s1 + h))(
                np.where(in0 >= 0, s0, -s0))),
    ))
    return xprep, wprep, wprep2


def _build_fast(inv_se, c0s, c1s):
    """Fast-path per-core Bass module (uniform post_bin_scale).

    inv_se, c0s, c1s are global scalars baked into the DVE ops.
    """
    key = ("fast", float(inv_se), float(c0s), float(c1s))
    if key in _CACHE:
        return _CACHE[key]

    import concourse.mybir as mybir
    import concourse.tile as tile
    from concourse import bacc

    xprep, _, wprep2 = _register_ops()

    nc = bacc.Bacc(None, target_bir_lowering=False)
    bf16 = mybir.dt.bfloat16
    f32 = mybir.dt.float32
    ident = mybir.ActivationFunctionType.Identity

    x_in = nc.dram_tensor("x", [NB_I, P, N_SHARD], f32, kind="ExternalInput")
    w_in = nc.dram_tensor("w", [NB_I, P, D_OUT], f32, kind="ExternalInput")
    m_in = nc.dram_tensor("m", [NB_I, P, D_OUT], mybir.dt.uint8,
                          kind="ExternalInput")
    fs_in = nc.dram_tensor("fs", [P, NB_O], f32, kind="ExternalInput")
    fb_in = nc.dram_tensor("fb", [P, NB_O], f32, kind="ExternalInput")
    out_o = nc.dram_tensor("out", [D_OUT, N_SHARD], bf16, kind="ExternalOutput")

    KH = NB_I // 2                  # contraction split: 8 + 8 k-steps
    # wave order matches operand production order: (nt0,ob0-7) needs only the
    # Xa/Wa halves, (nt1,ob0-7) adds Xb, (nt0,ob8-15) adds Wb, (nt1,ob8-15)
    # needs everything -- so no wave ever waits on prep at a phase boundary
    tiles = ([(ob, 0) for ob in range(8)] + [(ob, 1) for ob in range(8)]
             + [(ob, 0) for ob in range(8, 16)]
             + [(ob, 1) for ob in range(8, 16)])
    waves = [tiles[w0:w0 + 8] for w0 in range(0, len(tiles), 8)]

    with tile.TileContext(nc) as tc:
        with (
            tc.tile_pool(name="persist", bufs=1) as persist,
            tc.tile_pool(name="wlp", bufs=6) as wlp,
            tc.tile_pool(name="mlp", bufs=6) as mlp,
            tc.tile_pool(name="xlp", bufs=5) as xlp,
            tc.tile_pool(name="ostage", bufs=8) as ostage,
            tc.tile_pool(name="psum", bufs=8, space="PSUM") as psum,
        ):
            w2 = persist.tile([P, NB_I, D_OUT], bf16, tag="w2")   # [i_in, ib, o]
            xb = persist.tile([P, NB_I, N_SHARD], bf16, tag="xb")  # [i_in, ib, n]
            # bf16 partial sums for the first contraction half (PSUM is only
            # 8 banks; parking partials in SBUF lets all 32 output tiles
            # overlap the operand-prep phase)
            parts = persist.tile([P, len(tiles), 512], bf16, tag="parts")
            fs_sb = persist.tile([P, NB_O], f32, tag="fs")
            fb_sb = persist.tile([P, NB_O], f32, tag="fb")

            # PE warm-up source: a zeroed bf16 tile for dummy matmuls that
            # keep the tensor engine busy through the prep-latency window so
            # real matmuls start at full p-state.
            warm = persist.tile([P, 512], bf16, tag="warm")
            nc.vector.memset(warm[:], 0.0)

            # Operand prep in dependency order. Phase-1 wave 1 (nt=0, ob 0-7)
            # consumes one (Xa, Wa) half-block pair per 1.7 us; emitting
            # exactly those halves first makes DVE production match PE
            # consumption, eliminating wave-1 pacing stalls. Remaining halves
            # follow in the order later waves need them.
            XHF, WHF = N_SHARD // 2, D_OUT // 2

            def x_op(ib, lo, sz, hwdge=False):
                # hwdge=True: plain f32 load on the sync HWDGE ring. Used in
                # the pacing-critical first phase so the Pool engine's SWDGE
                # desc-gen (994ns fixed per DMA) only carries the w loads and
                # operand production keeps up with wave-1 consumption.
                if hwdge:
                    xt = xlp.tile([P, sz], f32, name="xt", tag="x_f32")
                    nc.sync.dma_start(xt[:],
                                      x_in[ib, :, lo:lo + sz])
                else:
                    xt = xlp.tile([P, sz], bf16, name="xt", tag="x_bf16")
                    nc.gpsimd.dma_start(xt[:],
                                        x_in[ib, :, lo:lo + sz])
                nc.vector._custom_dve(
                    xprep, out=xb[:, ib, lo:lo + sz], in0=xt[:],
                    s0=float(inv_se), s1=MAGIC, imm2=QMAX)

            def w_op(ib, lo, sz):
                mt = mlp.tile([P, sz], mybir.dt.uint8, name="mt", tag="m_u8")
                nc.sync.dma_start(mt[:], m_in[ib, :, lo:lo + sz])
                wt = wlp.tile([P, sz], bf16, name="wt", tag="w_bf16")
                nc.gpsimd.dma_start(wt[:], w_in[ib, :, lo:lo + sz])
                nc.vector._custom_dve(
                    wprep2, out=w2[:, ib, lo:lo + sz], in0=wt[:], in1=mt[:],
                    s0=float(0.5 * c0s), s1=float(c1s))

            # Wave 1 deps, paced 1:1. Block 0 takes minimum-latency load
            # paths: its x half rides the sync HWDGE ring as plain f32 (no
            # Pool desc-gen), making the first w load the Pool generator's
            # FIRST job -- the first matmul starts ~1us sooner. Blocks 1-7's
            # x halves load two per SWDGE DMA (one desc-gen each),
            # prefetched ahead of use.
            xt0 = xlp.tile([P, XHF], f32, name="xt0", tag="x0_f32")
            nc.sync.dma_start(xt0[:], x_in[0, :, 0:XHF])
            mt0 = mlp.tile([P, WHF], mybir.dt.uint8, name="mt0", tag="m_u8")
            nc.sync.dma_start(mt0[:], m_in[0, :, 0:WHF])
            wt0 = wlp.tile([P, WHF], bf16, name="wt0", tag="w_bf16")
            nc.gpsimd.dma_start(wt0[:], w_in[0, :, 0:WHF])

            xa_src = {}

            def xa_load(ib0, nb):
                t2 = xlp.tile([P, nb, XHF], bf16, name="t2", tag="x_bf16p")
                nc.gpsimd.dma_start(
                    t2[:], x_in[ib0:ib0 + nb, :, 0:XHF].transpose([1, 0, 2]))
                for j in range(nb):
                    xa_src[ib0 + j] = t2[:, j, :]

            xa_load(1, 2)
            nc.vector._custom_dve(
                xprep, out=xb[:, 0, 0:XHF], in0=xt0[:],
                s0=float(inv_se), s1=MAGIC, imm2=QMAX)
            # block-0 W half in two quarters: the first matmuls (ob 0-3)
            # need only w2[:, 0, 0:512], so they start ~0.5us sooner
            WQ = WHF // 2
            for q in range(2):
                nc.vector._custom_dve(
                    wprep2, out=w2[:, 0, q * WQ:(q + 1) * WQ],
                    in0=wt0[:, q * WQ:(q + 1) * WQ],
                    in1=mt0[:, q * WQ:(q + 1) * WQ],
                    s0=float(0.5 * c0s), s1=float(c1s))
            batches = [(3, 2), (5, 2), (7, 1)]
            bi = 0
            for ib in range(1, KH):
                nc.vector._custom_dve(
                    xprep, out=xb[:, ib, 0:XHF], in0=xa_src[ib],
                    s0=float(inv_se), s1=MAGIC, imm2=QMAX)
                w_op(ib, 0, WHF)
                if ib % 2 == 0 and bi < len(batches):
                    xa_load(*batches[bi])
                    bi += 1
            for ib in range(KH):                 # wave 2 deps (nt1, ob 0-7)
                x_op(ib, XHF, XHF)
            for ib in range(KH):                 # wave 3 deps (nt0, ob 8-15)
                w_op(ib, WHF, WHF)
            for ib in range(KH, NB_I):           # second contraction half
                x_op(ib, 0, N_SHARD)
                w_op(ib, 0, D_OUT)

            # epilogue constants (needed only after the first full tiles
            # finish, so loaded after the prep stream is underway)
            nc.scalar.dma_start(fs_sb[:], fs_in[:])
            nc.scalar.dma_start(fb_sb[:], fb_in[:])

            def mm(ps, ob, nt, ib, start, stop):
                nc.tensor.matmul(
                    ps[:],
                    w2[:, ib, ob * P:(ob + 1) * P],
                    xb[:, ib, nt * 512:(nt + 1) * 512],
                    start=start, stop=stop, skip_group_check=True)

            # phase 1 (overlaps prep): first contraction half for every tile,
            # k-outer within each 8-tile wave; partials parked in SBUF bf16.
            # The last wave's final 4 tiles keep their banks: phase 2
            # continues them in place while the first reloads pipeline in.
            held = []
            for wi, wave in enumerate(waves):
                pss = [psum.tile([P, 512], f32, name="ps", tag="ps")
                       for _ in wave]
                if wi == 0:
                    # dummies into pss[0]; the real ib-0 matmul resets it
                    # (start=True), so only timing is affected.
                    for _ in range(7):
                        nc.tensor.matmul(pss[0][:], warm[:, 0:P], warm[:],
                                         start=True, stop=True,
                                         skip_group_check=True)
                for ib in range(KH):
                    for ps, (ob, nt) in zip(pss, wave):
                        mm(ps, ob, nt, ib, ib == 0, ib == KH - 1)
                if wi == len(waves) - 1:
                    for k in range(4):
                        nc.scalar.copy(parts[:, wi * 8 + k, :], pss[k][:])
                    held = pss[4:]
                else:
                    for k, ps in enumerate(pss):
                        nc.scalar.copy(parts[:, wi * 8 + k, :], ps[:])

            def epilogue(ps, ob, nt):
                osb = ostage.tile([P, 512], bf16, tag="osb")
                nc.scalar.activation(
                    osb[:], ps[:], ident,
                    bias=fb_sb[:, ob:ob + 1], scale=fs_sb[:, ob:ob + 1])
                nc.sync.dma_start(
                    out_o[ob * P:(ob + 1) * P, nt * 512:(nt + 1) * 512],
                    osb[:])

            # phase 2: finish the 4 held tiles first (banks already hot) while
            # the first reload wave pipelines in on the Act queue; then parked
            # tiles in small waves, reloads running ahead of epilogues.
            parked = tiles[:28]
            held_tiles = tiles[28:]
            sizes = [4, 4, 4, 4, 4, 4, 2, 1, 1]
            p2, pos = [], 0
            for sz in sizes:
                p2.append(list(enumerate(parked[pos:pos + sz], start=pos)))
                pos += sz

            live_half = []

            def load_wave(wi):
                if wi == len(p2) - 1 and len(p2[wi]) == 1:
                    idx = p2[wi][0][0]
                    psA = psum.tile([P, 256], f32, name="psA", tag="ps")
                    psB = psum.tile([P, 256], f32, name="psB", tag="ps")
                    nc.scalar.copy(psA[:], parts[:, idx, 0:256])
                    nc.scalar.copy(psB[:], parts[:, idx, 256:512])
                    live_half.append(psB)
                    return [psA]
                pss = [psum.tile([P, 512], f32, name="ps", tag="ps")
                       for _ in p2[wi]]
                for ps, (idx, _) in zip(pss, p2[wi]):
                    nc.scalar.copy(ps[:], parts[:, idx, :])
                return pss

            live = {0: load_wave(0)}
            for ib in range(KH, NB_I):
                for ps, (ob, nt) in zip(held, held_tiles):
                    mm(ps, ob, nt, ib, False, ib == NB_I - 1)
            for ps, (ob, nt) in zip(held, held_tiles):
                epilogue(ps, ob, nt)
            live[1] = load_wave(1)

            for wi, wave in enumerate(p2):
                pss = live.pop(wi)
                if wi == len(p2) - 1 and len(wave) == 1:
                    # final tile in two independent 256-col halves: half A's
                    # epilogue+store launch while half B's matmuls run, and
                    # the closing drain chain shrinks to 256-wide ops
                    idx, (ob, nt) = wave[0]
                    psA, psB = pss[0], live_half[0]
                    for half, ps in ((0, psA), (1, psB)):
                        lo = nt * 512 + half * 256
                        for ib in range(KH, NB_I):
                            nc.tensor.matmul(
                                ps[:], w2[:, ib, ob * P:(ob + 1) * P],
                                xb[:, ib, lo:lo + 256],
                                start=False, stop=(ib == NB_I - 1),
                                skip_group_check=True)
                        osb = ostage.tile([P, 256], bf16, name="osbh",
                                          tag="osbh")
                        nc.scalar.activation(
                            osb[:], ps[:], ident,
                            bias=fb_sb[:, ob:ob + 1], scale=fs_sb[:, ob:ob + 1])
                        nc.sync.dma_start(
                            out_o[ob * P:(ob + 1) * P, lo:lo + 256], osb[:])
                    continue
                for ib in range(KH, NB_I):
                    for ps, (_, (ob, nt)) in zip(pss, wave):
                        mm(ps, ob, nt, ib, False, ib == NB_I - 1)
                for ps, (_, (ob, nt)) in zip(pss, wave):
                    epilogue(ps, ob, nt)
                if wi + 2 < len(p2):
                    live[wi + 2] = load_wave(wi + 2)

    nc.compile()
    _CACHE[key] = nc
    return nc


def _fast_consts(post_bin_scale, final_scale, final_bias, running_max):
    s = np.float32(running_max) / np.float32(QMAX)
    inv_se = np.float32(1.0) / (s + np.float32(EPS))
    sigma = np.float64(0.5) * (np.float64(s) + np.float64(EPS))
    pbs0 = np.float64(post_bin_scale.reshape(-1)[0])
    c0s = np.float32(sigma * pbs0)
    c1s = np.float32(np.float64(0.5) * sigma)
    fscol = np.ascontiguousarray(
        final_scale.astype(np.float32).reshape(NB_O, P).T)
    fbcol = np.ascontiguousarray(
        final_bias.astype(np.float32).reshape(NB_O, P).T)
    return inv_se, c0s, c1s, fscol, fbcol


def _fast_in_maps(x, weight, mask, fscol, fbcol):
    wT = np.ascontiguousarray(weight.T).reshape(NB_I, P, D_OUT)
    mT = np.ascontiguousarray(mask.T).view(np.uint8).reshape(NB_I, P, D_OUT)
    maps = []
    for c in range(N_CORES):
        xT = np.ascontiguousarray(x[c * N_SHARD:(c + 1) * N_SHARD].T)
        maps.append({
            "x": xT.reshape(NB_I, P, N_SHARD),
            "w": wT,
            "m": mT,
            "fs": fscol,
            "fb": fbcol,
        })
    return maps


# ---------------------------------------------------------------------------
# general fallback (non-uniform post_bin_scale): previous implementation
# ---------------------------------------------------------------------------

def _build_general(inv_se):
    key = ("nc", float(inv_se))
    if key in _CACHE:
        return _CACHE[key]

    import concourse.mybir as mybir
    import concourse.tile as tile
    from concourse import bacc

    xprep, wprep, _ = _register_ops()

    nc = bacc.Bacc(None, target_bir_lowering=False)
    bf16 = mybir.dt.bfloat16
    f32 = mybir.dt.float32

    x_in = nc.dram_tensor("x", [N_SHARD, D_IN], f32, kind="ExternalInput")
    w_in = nc.dram_tensor("w", [D_OUT, D_IN], f32, kind="ExternalInput")
    m_in = nc.dram_tensor("m", [D_OUT, D_IN], mybir.dt.uint8, kind="ExternalInput")
    c0_in = nc.dram_tensor("c0", [P, NB_O], f32, kind="ExternalInput")
    c1_in = nc.dram_tensor("c1", [P, NB_O], f32, kind="ExternalInput")
    fb_in = nc.dram_tensor("fb", [P, D_OUT], f32, kind="ExternalInput")
    out_o = nc.dram_tensor("out", [N_SHARD, D_OUT], f32, kind="ExternalOutput")

    from concourse.masks import make_identity

    with tile.TileContext(nc) as tc:
        with (
            tc.tile_pool(name="persist", bufs=1) as persist,
            tc.tile_pool(name="wlp", bufs=4) as wlp,
            tc.tile_pool(name="wpp", bufs=4) as wpp,
            tc.tile_pool(name="xlp", bufs=4) as xlp,
            tc.tile_pool(name="xbp", bufs=4) as xbp,
            tc.tile_pool(name="ostage", bufs=7) as ostage,
            tc.tile_pool(name="psum", bufs=6, space="PSUM") as psum,
            tc.tile_pool(name="tpsum", bufs=2, space="PSUM") as tpsum,
        ):
            wT = persist.tile([P, NB_O, NB_I, P], bf16, tag="wT")
            xqT = persist.tile([P, NJ, NB_I, P], bf16, tag="xqT")
            c0_sb = persist.tile([P, NB_O], f32, tag="c0")
            c1_sb = persist.tile([P, NB_O], f32, tag="c1")
            fb_sb = persist.tile([P, D_OUT], f32, tag="fb")
            ident = persist.tile([P, P], bf16, tag="ident")

            nc.sync.dma_start(fb_sb[:], fb_in[:])
            nc.sync.dma_start(c0_sb[:], c0_in[:])
            nc.sync.dma_start(c1_sb[:], c1_in[:])
            make_identity(nc, ident[:])

            def w_block(ob):
                wt = wlp.tile([P, D_IN], bf16, tag="w_bf16")
                mt = wlp.tile([P, D_IN], mybir.dt.uint8, tag="m_u8")
                nc.gpsimd.dma_start(wt[:], w_in[ob * P:(ob + 1) * P, :])
                nc.scalar.dma_start(mt[:], m_in[ob * P:(ob + 1) * P, :])
                w2 = wpp.tile([P, D_IN], bf16, tag="w2")
                nc.vector._custom_dve(
                    wprep, out=w2[:], in0=wt[:], in1=mt[:],
                    s0=c0_sb[:, ob:ob + 1], s1=c1_sb[:, ob:ob + 1], imm2=0.5)
                nc.sync.dma_start_transpose(wT[:, ob], w2[:])

            def x_block(j):
                xt = xlp.tile([P, D_IN], f32, tag="x_f32")
                nc.sync.dma_start(xt[:], x_in[j * P:(j + 1) * P, :])
                xb = xbp.tile([P, D_IN], bf16, tag="xb")
                nc.vector._custom_dve(
                    xprep, out=xb[:], in0=xt[:],
                    s0=float(inv_se), s1=MAGIC, imm2=QMAX)
                for b in range(NB_I):
                    tp = tpsum.tile([P, P], bf16, tag="xtp")
                    nc.tensor.transpose(tp[:], xb[:, b * P:(b + 1) * P], ident[:])
                    nc.scalar.copy(xqT[:, j, b, :], tp[:])

            for ob in range(OB_PER_T):
                w_block(ob)
            for j in range(NJ):
                x_block(j)
            for ob in range(OB_PER_T, NB_O):
                w_block(ob)

            for t in range(NT):
                for j in range(NJ):
                    ps = psum.tile([P, OT], f32, tag="ps")
                    for b in range(NB_I):
                        nc.tensor.matmul(
                            ps[:],
                            xqT[:, j, b, :],
                            wT[:, t * OB_PER_T:(t + 1) * OB_PER_T, b, :],
                            start=(b == 0), stop=(b == NB_I - 1))
                    osb = ostage.tile([P, OT], f32, tag="osb")
                    nc.vector.tensor_add(
                        osb[:], ps[:], fb_sb[:, t * OT:(t + 1) * OT])
                    nc.scalar.dma_start(
                        out_o[j * P:(j + 1) * P, t * OT:(t + 1) * OT], osb[:])

    nc.compile()
    _CACHE[key] = nc
    return nc


def _general_in_maps(x, weight, mask_u8, c0, c1, fb):
    maps = []
    for c in range(N_CORES):
        maps.append({
            "x": np.ascontiguousarray(x[c * N_SHARD:(c + 1) * N_SHARD]),
            "w": weight,
            "m": mask_u8,
            "c0": c0,
            "c1": c1,
            "fb": fb,
        })
    return maps


def _general_consts(post_bin_scale, final_scale, final_bias, running_max):
    s = np.float32(running_max) / np.float32(QMAX)
    inv_se = np.float32(1.0) / (s + np.float32(EPS))
    sigma = np.float64(0.5) * (np.float64(s) + np.float64(EPS))
    c0_all = (sigma * final_scale.astype(np.float64)
              * post_bin_scale.reshape(-1).astype(np.float64)).astype(np.float32)
    c1_all = (np.float64(0.5) * sigma
              * final_scale.astype(np.float64)).astype(np.float32)
    c0 = np.ascontiguousarray(c0_all.reshape(NB_O, P).T)
    c1 = np.ascontiguousarray(c1_all.reshape(NB_O, P).T)
    fb = np.ascontiguousarray(
        np.broadcast_to(final_bias.astype(np.float32), (P, D_OUT)))
    return inv_se, c0, c1, fb


def _run_spmd(nc, maps):
    """Execute with retry: axon-tunneled devices can transiently fail."""
    from concourse.bass_utils import run_bass_kernel_spmd
    for attempt in range(3):
        try:
            return run_bass_kernel_spmd(nc, maps, core_ids=list(range(N_CORES)))
        except Exception:  # noqa: BLE001 - retrying device-side faults
            if attempt == 2:
                raise
            import gc
            import time as _time
            gc.collect()
            try:
                import jax
                jax.clear_caches()
                import jax.extend as _jex
                _jex.backend.clear_backends()
            except Exception:
                pass
            _time.sleep(10)


def prepare(x, weight, post_bin_scale, final_scale, final_bias, running_max,
            sprinkle_mask):
    """Build (compile) the module and the per-core input maps."""
    x = np.asarray(x, dtype=np.float32)
    weight = np.ascontiguousarray(np.asarray(weight, dtype=np.float32))
    mask = np.asarray(sprinkle_mask)
    pbs = np.asarray(post_bin_scale, dtype=np.float32)
    fs = np.asarray(final_scale, dtype=np.float32)
    fb = np.asarray(final_bias, dtype=np.float32)
    rm = float(np.asarray(running_max))

    if np.all(pbs.reshape(-1) == pbs.reshape(-1)[0]):
        inv_se, c0s, c1s, fscol, fbcol = _fast_consts(pbs, fs, fb, rm)
        nc = _build_fast(inv_se, c0s, c1s)
        maps = _fast_in_maps(x, weight, mask, fscol, fbcol)
        fast = True
    else:
        inv_se, c0, c1, fbb = _general_consts(pbs, fs, fb, rm)
        nc = _build_general(inv_se)
        maps = _general_in_maps(
            x, weight, np.ascontiguousarray(mask).view(np.uint8), c0, c1, fbb)
        fast = False
    return nc, maps, fast


def kernel(x, weight, post_bin_scale, final_scale, final_bias, running_max,
           sprinkle_mask):
    nc, maps, fast = prepare(x, weight, post_bin_scale, final_scale,
                             final_bias, running_max, sprinkle_mask)
    res = _run_spmd(nc, maps)
    if fast:
        out = np.concatenate(
            [np.asarray(res.results[c]["out"]).astype(np.float32).T
             for c in range(N_CORES)], axis=0)
    else:
        out = np.concatenate(
            [res.results[c]["out"] for c in range(N_CORES)], axis=0)
    return np.ascontiguousarray(out)


# revision 30
# speedup vs baseline: 1.4693x; 1.0009x over previous
"""BitLinear (quantized-activation, binarized-sprinkled-weight linear) Trainium2 kernel.

Data-parallel over the token dim N across 8 NeuronCores.

Fast path (post_bin_scale uniform, which holds for this problem's inputs):
  * all operands are laid out HOST-side (pure transposes/views) so the
    contraction dim i lands on SBUF partitions with no device transposes:
       xT [IN, N_SHARD] f32 (per core),  wT [IN, OUT] f32,  mT [IN, OUT] u8
  * w/x are DMA-cast to bf16 on load (gpsimd SWDGE); one fused DVE op per
    operand block:
       xb = t + clip(round(t), +-127),  t = x*inv_se          (bf16)
       W2 = m ? sign(w)*C0 : w*C1 + sign(w)*C0*0.5            (bf16)
    with C0 = sigma*pbs, C1 = 0.5*sigma global scalars (sigma = 0.5*(s+eps)),
    so that  out[n,o] = (xb @ W2^T)*fs[o] + fb[o]
  * prep is emitted in dependency order at half-block granularity so DVE
    production (the prep bottleneck, custom ops run 1x) matches the PE's
    PSUM-capacity-limited consumption exactly -- no matmul pacing stalls
  * matmuls produce PSUM tiles [o=128, n=512] (o on partitions), 16 k-steps.
    PSUM only holds 8 such tiles, so the contraction is split 8+8: during
    prep, every output tile's first half accumulates in waves of 8 and is
    parked in SBUF as bf16 partials; after prep the halves are reloaded
    (Act engine, software-pipelined two waves deep) and finished
  * epilogue on the Activation engine: Identity with per-partition scale=fs,
    bias=fb, writing bf16; stores go out transposed [OUT, N_SHARD] and the
    host transposes back / upcasts (layout-only work).

General path (non-uniform post_bin_scale): previous per-partition-constant
implementation, kept verbatim as a fallback.
"""

import numpy as np

N_CORES = 8
N_TOK, D_IN, D_OUT = 8192, 2048, 2048
N_SHARD = N_TOK // N_CORES          # 1024 rows of x per core
P = 128
NJ = N_SHARD // P                   # 8 n-blocks per core
NB_I = D_IN // P                    # 16 i-blocks (contraction)
NB_O = D_OUT // P                   # 16 o-blocks
OT = 512                            # o-tile (one PSUM bank)
NT = D_OUT // OT                    # 4 o-tiles
OB_PER_T = OT // P                  # 4 o-blocks per o-tile

MM_NT = N_SHARD // 512              # fast path: 2 moving-dim tiles of 512

QMAX = 127.0
EPS = 1e-6
MAGIC = 12582912.0                  # 1.5 * 2**23: fp32 RNE round-to-int trick

_CACHE = {}


def _register_ops():
    """Define the two fused DVE ops (idempotent)."""
    from concourse import dve_ops
    from concourse.dve_spec import (
        Spec, Src0, Src1, C0, C1, C2, Zero, select, minn, maxx, lower, _has_src1,
    )
    from concourse.dve_uop import DveOpSpec

    def register(name, spec):
        for op in dve_ops.OPS:
            if op.name == name:
                return op
        ver = "v3"
        tmp = DveOpSpec(name=name, opcode=0, uops=lower(spec, ver=ver),
                        rd1_en=_has_src1(spec))
        op = dve_ops.DveOp(name, spec, subdim=False,
                           uops_sha={ver: tmp.sha(ver)})
        dve_ops.OPS.append(op)
        dve_ops._SUB_OPCODE_FOR_NAME[name] = (
            max(dve_ops._SUB_OPCODE_FOR_NAME.values()) + 1)
        dve_ops.CUSTOM_DVE_SPECS[name] = spec
        return op

    # out = t + clip(round(t), +-imm2), t = x*s0   (s1 = MAGIC)
    _t = Src0 * C0
    _r = (_t + C1) - C1
    _rc = minn(maxx(_r, Zero - C2), C2)
    xprep = register("XPREP_BITLIN", Spec(
        body=_t + _rc,
        reference=lambda in0, in1, s0, s1, imm2: (
            (lambda t: t + np.clip(np.round(t), -imm2, imm2))(
                in0.astype(np.float32) * s0)),
    ))

    # h = select(w>=0, s0, -s0); out = select(m>0, h, w*s1 + h*imm2)
    _h = select(Src0 >= Zero, C0, Zero - C0)
    wprep = register("WPREP_BITLIN", Spec(
        body=select(Src1 > Zero, _h, Src0 * C1 + _h * C2),
        reference=lambda in0, in1, s0, s1, imm2: (
            (lambda h: np.where(in1 > 0, h,
                                in0.astype(np.float32) * s1 + h * imm2))(
                np.where(in0 >= 0, s0, -s0))),
    ))

    # two-constant variant (no imm2 -> allows 2-free-dim src1):
    # hh = select(w>=0, s0, -s0); out = select(m>0, hh+hh, w*s1 + hh)
    # with s0 = 0.5*C0 so that hh+hh = sign(w)*C0.
    _hh = select(Src0 >= Zero, C0, Zero - C0)
    wprep2 = register("WPREP2_BITLIN", Spec(
        body=select(Src1 > Zero, _hh + _hh, Src0 * C1 + _hh),
        reference=lambda in0, in1, s0, s1, imm2: (
            (lambda h: np.where(in1 > 0, h + h,
                                in0.astype(np.float32) * s1 + h))(
                np.where(in0 >= 0, s0, -s0))),
    ))
    return xprep, wprep, wprep2


def _build_fast(inv_se, c0s, c1s):
    """Fast-path per-core Bass module (uniform post_bin_scale).

    inv_se, c0s, c1s are global scalars baked into the DVE ops.
    """
    key = ("fast", float(inv_se), float(c0s), float(c1s))
    if key in _CACHE:
        return _CACHE[key]

    import concourse.mybir as mybir
    import concourse.tile as tile
    from concourse import bacc

    xprep, _, wprep2 = _register_ops()

    nc = bacc.Bacc(None, target_bir_lowering=False)
    bf16 = mybir.dt.bfloat16
    f32 = mybir.dt.float32
    ident = mybir.ActivationFunctionType.Identity

    x_in = nc.dram_tensor("x", [NB_I, P, N_SHARD], f32, kind="ExternalInput")
    w_in = nc.dram_tensor("w", [NB_I, P, D_OUT], f32, kind="ExternalInput")
    m_in = nc.dram_tensor("m", [NB_I, P, D_OUT], mybir.dt.uint8,
                          kind="ExternalInput")
    fs_in = nc.dram_tensor("fs", [P, NB_O], f32, kind="ExternalInput")
    fb_in = nc.dram_tensor("fb", [P, NB_O], f32, kind="ExternalInput")
    out_o = nc.dram_tensor("out", [D_OUT, N_SHARD], bf16, kind="ExternalOutput")

    KH = NB_I // 2                  # contraction split: 8 + 8 k-steps
    # wave order matches operand production order: (nt0,ob0-7) needs only the
    # Xa/Wa halves, (nt1,ob0-7) adds Xb, (nt0,ob8-15) adds Wb, (nt1,ob8-15)
    # needs everything -- so no wave ever waits on prep at a phase boundary
    tiles = ([(ob, 0) for ob in range(8)] + [(ob, 1) for ob in range(8)]
             + [(ob, 0) for ob in range(8, 16)]
             + [(ob, 1) for ob in range(8, 16)])
    waves = [tiles[w0:w0 + 8] for w0 in range(0, len(tiles), 8)]

    with tile.TileContext(nc) as tc:
        with (
            tc.tile_pool(name="persist", bufs=1) as persist,
            tc.tile_pool(name="wlp", bufs=6) as wlp,
            tc.tile_pool(name="mlp", bufs=6) as mlp,
            tc.tile_pool(name="xlp", bufs=5) as xlp,
            tc.tile_pool(name="ostage", bufs=8) as ostage,
            tc.tile_pool(name="psum", bufs=8, space="PSUM") as psum,
        ):
            w2 = persist.tile([P, NB_I, D_OUT], bf16, tag="w2")   # [i_in, ib, o]
            xb = persist.tile([P, NB_I, N_SHARD], bf16, tag="xb")  # [i_in, ib, n]
            # bf16 partial sums for the first contraction half (PSUM is only
            # 8 banks; parking partials in SBUF lets all 32 output tiles
            # overlap the operand-prep phase)
            parts = persist.tile([P, len(tiles), 512], bf16, tag="parts")
            fs_sb = persist.tile([P, NB_O], f32, tag="fs")
            fb_sb = persist.tile([P, NB_O], f32, tag="fb")

            # PE warm-up source: a zeroed bf16 tile for dummy matmuls that
            # keep the tensor engine busy through the prep-latency window so
            # real matmuls start at full p-state.
            warm = persist.tile([P, 512], bf16, tag="warm")
            nc.vector.memset(warm[:], 0.0)

            # Operand prep in dependency order. Phase-1 wave 1 (nt=0, ob 0-7)
            # consumes one (Xa, Wa) half-block pair per 1.7 us; emitting
            # exactly those halves first makes DVE production match PE
            # consumption, eliminating wave-1 pacing stalls. Remaining halves
            # follow in the order later waves need them.
            XHF, WHF = N_SHARD // 2, D_OUT // 2

            def x_op(ib, lo, sz, hwdge=False):
                # hwdge=True: plain f32 load on the sync HWDGE ring. Used in
                # the pacing-critical first phase so the Pool engine's SWDGE
                # desc-gen (994ns fixed per DMA) only carries the w loads and
                # operand production keeps up with wave-1 consumption.
                if hwdge:
                    xt = xlp.tile([P, sz], f32, name="xt", tag="x_f32")
                    nc.sync.dma_start(xt[:],
                                      x_in[ib, :, lo:lo + sz])
                else:
                    xt = xlp.tile([P, sz], bf16, name="xt", tag="x_bf16")
                    nc.gpsimd.dma_start(xt[:],
                                        x_in[ib, :, lo:lo + sz])
                nc.vector._custom_dve(
                    xprep, out=xb[:, ib, lo:lo + sz], in0=xt[:],
                    s0=float(inv_se), s1=MAGIC, imm2=QMAX)

            def w_op(ib, lo, sz):
                mt = mlp.tile([P, sz], mybir.dt.uint8, name="mt", tag="m_u8")
                nc.sync.dma_start(mt[:], m_in[ib, :, lo:lo + sz])
                wt = wlp.tile([P, sz], bf16, name="wt", tag="w_bf16")
                nc.gpsimd.dma_start(wt[:], w_in[ib, :, lo:lo + sz])
                nc.vector._custom_dve(
                    wprep2, out=w2[:, ib, lo:lo + sz], in0=wt[:], in1=mt[:],
                    s0=float(0.5 * c0s), s1=float(c1s))

            # Wave 1 deps, paced 1:1. Block 0 takes minimum-latency load
            # paths: its x half rides the sync HWDGE ring as plain f32 (no
            # Pool desc-gen), making the first w load the Pool generator's
            # FIRST job -- the first matmul starts ~1us sooner. Blocks 1-7's
            # x halves load two per SWDGE DMA (one desc-gen each),
            # prefetched ahead of use.
            xt0 = xlp.tile([P, XHF], f32, name="xt0", tag="x0_f32")
            nc.sync.dma_start(xt0[:], x_in[0, :, 0:XHF])
            wt0 = wlp.tile([P, WHF], bf16, name="wt0", tag="w_bf16")
            nc.gpsimd.dma_start(wt0[:], w_in[0, :, 0:WHF])
            mt0 = mlp.tile([P, WHF], mybir.dt.uint8, name="mt0", tag="m_u8")
            nc.sync.dma_start(mt0[:, 0:WHF // 2], m_in[0, :, 0:WHF // 2])
            nc.sync.dma_start(mt0[:, WHF // 2:], m_in[0, :, WHF // 2:WHF])

            xa_src = {}

            def xa_load(ib0, nb):
                t2 = xlp.tile([P, nb, XHF], bf16, name="t2", tag="x_bf16p")
                nc.gpsimd.dma_start(
                    t2[:], x_in[ib0:ib0 + nb, :, 0:XHF].transpose([1, 0, 2]))
                for j in range(nb):
                    xa_src[ib0 + j] = t2[:, j, :]

            xa_load(1, 2)
            nc.vector._custom_dve(
                xprep, out=xb[:, 0, 0:XHF], in0=xt0[:],
                s0=float(inv_se), s1=MAGIC, imm2=QMAX)
            # block-0 W half in two quarters: the first matmuls (ob 0-3)
            # need only w2[:, 0, 0:512], so they start ~0.5us sooner
            WQ = WHF // 2
            for q in range(2):
                nc.vector._custom_dve(
                    wprep2, out=w2[:, 0, q * WQ:(q + 1) * WQ],
                    in0=wt0[:, q * WQ:(q + 1) * WQ],
                    in1=mt0[:, q * WQ:(q + 1) * WQ],
                    s0=float(0.5 * c0s), s1=float(c1s))
            batches = [(3, 2), (5, 2), (7, 1)]
            bi = 0
            for ib in range(1, KH):
                nc.vector._custom_dve(
                    xprep, out=xb[:, ib, 0:XHF], in0=xa_src[ib],
                    s0=float(inv_se), s1=MAGIC, imm2=QMAX)
                w_op(ib, 0, WHF)
                if ib % 2 == 0 and bi < len(batches):
                    xa_load(*batches[bi])
                    bi += 1
            for ib in range(KH):                 # wave 2 deps (nt1, ob 0-7)
                x_op(ib, XHF, XHF)
            for ib in range(KH):                 # wave 3 deps (nt0, ob 8-15)
                w_op(ib, WHF, WHF)
            for ib in range(KH, NB_I):           # second contraction half
                x_op(ib, 0, N_SHARD)
                w_op(ib, 0, D_OUT)

            # epilogue constants (needed only after the first full tiles
            # finish, so loaded after the prep stream is underway)
            nc.scalar.dma_start(fs_sb[:], fs_in[:])
            nc.scalar.dma_start(fb_sb[:], fb_in[:])

            def mm(ps, ob, nt, ib, start, stop):
                nc.tensor.matmul(
                    ps[:],
                    w2[:, ib, ob * P:(ob + 1) * P],
                    xb[:, ib, nt * 512:(nt + 1) * 512],
                    start=start, stop=stop, skip_group_check=True)

            # phase 1 (overlaps prep): first contraction half for every tile,
            # k-outer within each 8-tile wave; partials parked in SBUF bf16.
            # The last wave's final 4 tiles keep their banks: phase 2
            # continues them in place while the first reloads pipeline in.
            held = []
            for wi, wave in enumerate(waves):
                pss = [psum.tile([P, 512], f32, name="ps", tag="ps")
                       for _ in wave]
                if wi == 0:
                    # dummies into pss[0]; the real ib-0 matmul resets it
                    # (start=True), so only timing is affected.
                    for _ in range(7):
                        nc.tensor.matmul(pss[0][:], warm[:, 0:P], warm[:],
                                         start=True, stop=True,
                                         skip_group_check=True)
                for ib in range(KH):
                    for ps, (ob, nt) in zip(pss, wave):
                        mm(ps, ob, nt, ib, ib == 0, ib == KH - 1)
                if wi == len(waves) - 1:
                    for k in range(4):
                        nc.scalar.copy(parts[:, wi * 8 + k, :], pss[k][:])
                    held = pss[4:]
                else:
                    for k, ps in enumerate(pss):
                        nc.scalar.copy(parts[:, wi * 8 + k, :], ps[:])

            def epilogue(ps, ob, nt):
                osb = ostage.tile([P, 512], bf16, tag="osb")
                nc.scalar.activation(
                    osb[:], ps[:], ident,
                    bias=fb_sb[:, ob:ob + 1], scale=fs_sb[:, ob:ob + 1])
                nc.sync.dma_start(
                    out_o[ob * P:(ob + 1) * P, nt * 512:(nt + 1) * 512],
                    osb[:])

            # phase 2: finish the 4 held tiles first (banks already hot) while
            # the first reload wave pipelines in on the Act queue; then parked
            # tiles in small waves, reloads running ahead of epilogues.
            parked = tiles[:28]
            held_tiles = tiles[28:]
            sizes = [4, 4, 4, 4, 4, 4, 2, 1, 1]
            p2, pos = [], 0
            for sz in sizes:
                p2.append(list(enumerate(parked[pos:pos + sz], start=pos)))
                pos += sz

            live_half = []

            def load_wave(wi):
                if wi == len(p2) - 1 and len(p2[wi]) == 1:
                    idx = p2[wi][0][0]
                    psA = psum.tile([P, 256], f32, name="psA", tag="ps")
                    psB = psum.tile([P, 256], f32, name="psB", tag="ps")
                    nc.scalar.copy(psA[:], parts[:, idx, 0:256])
                    nc.scalar.copy(psB[:], parts[:, idx, 256:512])
                    live_half.append(psB)
                    return [psA]
                pss = [psum.tile([P, 512], f32, name="ps", tag="ps")
                       for _ in p2[wi]]
                for ps, (idx, _) in zip(pss, p2[wi]):
                    nc.scalar.copy(ps[:], parts[:, idx, :])
                return pss

            live = {0: load_wave(0)}
            for ib in range(KH, NB_I):
                for ps, (ob, nt) in zip(held, held_tiles):
                    mm(ps, ob, nt, ib, False, ib == NB_I - 1)
            for ps, (ob, nt) in zip(held, held_tiles):
                epilogue(ps, ob, nt)
            live[1] = load_wave(1)

            for wi, wave in enumerate(p2):
                pss = live.pop(wi)
                if wi == len(p2) - 1 and len(wave) == 1:
                    # final tile in two independent 256-col halves: half A's
                    # epilogue+store launch while half B's matmuls run, and
                    # the closing drain chain shrinks to 256-wide ops
                    idx, (ob, nt) = wave[0]
                    psA, psB = pss[0], live_half[0]
                    for half, ps in ((0, psA), (1, psB)):
                        lo = nt * 512 + half * 256
                        for ib in range(KH, NB_I):
                            nc.tensor.matmul(
                                ps[:], w2[:, ib, ob * P:(ob + 1) * P],
                                xb[:, ib, lo:lo + 256],
                                start=False, stop=(ib == NB_I - 1),
                                skip_group_check=True)
                        osb = ostage.tile([P, 256], bf16, name="osbh",
                                          tag="osbh")
                        nc.scalar.activation(
                            osb[:], ps[:], ident,
                            bias=fb_sb[:, ob:ob + 1], scale=fs_sb[:, ob:ob + 1])
                        nc.sync.dma_start(
                            out_o[ob * P:(ob + 1) * P, lo:lo + 256], osb[:])
                    continue
                for ib in range(KH, NB_I):
                    for ps, (_, (ob, nt)) in zip(pss, wave):
                        mm(ps, ob, nt, ib, False, ib == NB_I - 1)
                for ps, (_, (ob, nt)) in zip(pss, wave):
                    epilogue(ps, ob, nt)
                if wi + 2 < len(p2):
                    live[wi + 2] = load_wave(wi + 2)

    nc.compile()
    _CACHE[key] = nc
    return nc


def _fast_consts(post_bin_scale, final_scale, final_bias, running_max):
    s = np.float32(running_max) / np.float32(QMAX)
    inv_se = np.float32(1.0) / (s + np.float32(EPS))
    sigma = np.float64(0.5) * (np.float64(s) + np.float64(EPS))
    pbs0 = np.float64(post_bin_scale.reshape(-1)[0])
    c0s = np.float32(sigma * pbs0)
    c1s = np.float32(np.float64(0.5) * sigma)
    fscol = np.ascontiguousarray(
        final_scale.astype(np.float32).reshape(NB_O, P).T)
    fbcol = np.ascontiguousarray(
        final_bias.astype(np.float32).reshape(NB_O, P).T)
    return inv_se, c0s, c1s, fscol, fbcol


def _fast_in_maps(x, weight, mask, fscol, fbcol):
    wT = np.ascontiguousarray(weight.T).reshape(NB_I, P, D_OUT)
    mT = np.ascontiguousarray(mask.T).view(np.uint8).reshape(NB_I, P, D_OUT)
    maps = []
    for c in range(N_CORES):
        xT = np.ascontiguousarray(x[c * N_SHARD:(c + 1) * N_SHARD].T)
        maps.append({
            "x": xT.reshape(NB_I, P, N_SHARD),
            "w": wT,
            "m": mT,
            "fs": fscol,
            "fb": fbcol,
        })
    return maps


# ---------------------------------------------------------------------------
# general fallback (non-uniform post_bin_scale): previous implementation
# ---------------------------------------------------------------------------

def _build_general(inv_se):
    key = ("nc", float(inv_se))
    if key in _CACHE:
        return _CACHE[key]

    import concourse.mybir as mybir
    import concourse.tile as tile
    from concourse import bacc

    xprep, wprep, _ = _register_ops()

    nc = bacc.Bacc(None, target_bir_lowering=False)
    bf16 = mybir.dt.bfloat16
    f32 = mybir.dt.float32

    x_in = nc.dram_tensor("x", [N_SHARD, D_IN], f32, kind="ExternalInput")
    w_in = nc.dram_tensor("w", [D_OUT, D_IN], f32, kind="ExternalInput")
    m_in = nc.dram_tensor("m", [D_OUT, D_IN], mybir.dt.uint8, kind="ExternalInput")
    c0_in = nc.dram_tensor("c0", [P, NB_O], f32, kind="ExternalInput")
    c1_in = nc.dram_tensor("c1", [P, NB_O], f32, kind="ExternalInput")
    fb_in = nc.dram_tensor("fb", [P, D_OUT], f32, kind="ExternalInput")
    out_o = nc.dram_tensor("out", [N_SHARD, D_OUT], f32, kind="ExternalOutput")

    from concourse.masks import make_identity

    with tile.TileContext(nc) as tc:
        with (
            tc.tile_pool(name="persist", bufs=1) as persist,
            tc.tile_pool(name="wlp", bufs=4) as wlp,
            tc.tile_pool(name="wpp", bufs=4) as wpp,
            tc.tile_pool(name="xlp", bufs=4) as xlp,
            tc.tile_pool(name="xbp", bufs=4) as xbp,
            tc.tile_pool(name="ostage", bufs=7) as ostage,
            tc.tile_pool(name="psum", bufs=6, space="PSUM") as psum,
            tc.tile_pool(name="tpsum", bufs=2, space="PSUM") as tpsum,
        ):
            wT = persist.tile([P, NB_O, NB_I, P], bf16, tag="wT")
            xqT = persist.tile([P, NJ, NB_I, P], bf16, tag="xqT")
            c0_sb = persist.tile([P, NB_O], f32, tag="c0")
            c1_sb = persist.tile([P, NB_O], f32, tag="c1")
            fb_sb = persist.tile([P, D_OUT], f32, tag="fb")
            ident = persist.tile([P, P], bf16, tag="ident")

            nc.sync.dma_start(fb_sb[:], fb_in[:])
            nc.sync.dma_start(c0_sb[:], c0_in[:])
            nc.sync.dma_start(c1_sb[:], c1_in[:])
            make_identity(nc, ident[:])

            def w_block(ob):
                wt = wlp.tile([P, D_IN], bf16, tag="w_bf16")
                mt = wlp.tile([P, D_IN], mybir.dt.uint8, tag="m_u8")
                nc.gpsimd.dma_start(wt[:], w_in[ob * P:(ob + 1) * P, :])
                nc.scalar.dma_start(mt[:], m_in[ob * P:(ob + 1) * P, :])
                w2 = wpp.tile([P, D_IN], bf16, tag="w2")
                nc.vector._custom_dve(
                    wprep, out=w2[:], in0=wt[:], in1=mt[:],
                    s0=c0_sb[:, ob:ob + 1], s1=c1_sb[:, ob:ob + 1], imm2=0.5)
                nc.sync.dma_start_transpose(wT[:, ob], w2[:])

            def x_block(j):
                xt = xlp.tile([P, D_IN], f32, tag="x_f32")
                nc.sync.dma_start(xt[:], x_in[j * P:(j + 1) * P, :])
                xb = xbp.tile([P, D_IN], bf16, tag="xb")
                nc.vector._custom_dve(
                    xprep, out=xb[:], in0=xt[:],
                    s0=float(inv_se), s1=MAGIC, imm2=QMAX)
                for b in range(NB_I):
                    tp = tpsum.tile([P, P], bf16, tag="xtp")
                    nc.tensor.transpose(tp[:], xb[:, b * P:(b + 1) * P], ident[:])
                    nc.scalar.copy(xqT[:, j, b, :], tp[:])

            for ob in range(OB_PER_T):
                w_block(ob)
            for j in range(NJ):
                x_block(j)
            for ob in range(OB_PER_T, NB_O):
                w_block(ob)

            for t in range(NT):
                for j in range(NJ):
                    ps = psum.tile([P, OT], f32, tag="ps")
                    for b in range(NB_I):
                        nc.tensor.matmul(
                            ps[:],
                            xqT[:, j, b, :],
                            wT[:, t * OB_PER_T:(t + 1) * OB_PER_T, b, :],
                            start=(b == 0), stop=(b == NB_I - 1))
                    osb = ostage.tile([P, OT], f32, tag="osb")
                    nc.vector.tensor_add(
                        osb[:], ps[:], fb_sb[:, t * OT:(t + 1) * OT])
                    nc.scalar.dma_start(
                        out_o[j * P:(j + 1) * P, t * OT:(t + 1) * OT], osb[:])

    nc.compile()
    _CACHE[key] = nc
    return nc


def _general_in_maps(x, weight, mask_u8, c0, c1, fb):
    maps = []
    for c in range(N_CORES):
        maps.append({
            "x": np.ascontiguousarray(x[c * N_SHARD:(c + 1) * N_SHARD]),
            "w": weight,
            "m": mask_u8,
            "c0": c0,
            "c1": c1,
            "fb": fb,
        })
    return maps


def _general_consts(post_bin_scale, final_scale, final_bias, running_max):
    s = np.float32(running_max) / np.float32(QMAX)
    inv_se = np.float32(1.0) / (s + np.float32(EPS))
    sigma = np.float64(0.5) * (np.float64(s) + np.float64(EPS))
    c0_all = (sigma * final_scale.astype(np.float64)
              * post_bin_scale.reshape(-1).astype(np.float64)).astype(np.float32)
    c1_all = (np.float64(0.5) * sigma
              * final_scale.astype(np.float64)).astype(np.float32)
    c0 = np.ascontiguousarray(c0_all.reshape(NB_O, P).T)
    c1 = np.ascontiguousarray(c1_all.reshape(NB_O, P).T)
    fb = np.ascontiguousarray(
        np.broadcast_to(final_bias.astype(np.float32), (P, D_OUT)))
    return inv_se, c0, c1, fb


def _run_spmd(nc, maps):
    """Execute with retry: axon-tunneled devices can transiently fail."""
    from concourse.bass_utils import run_bass_kernel_spmd
    for attempt in range(3):
        try:
            return run_bass_kernel_spmd(nc, maps, core_ids=list(range(N_CORES)))
        except Exception:  # noqa: BLE001 - retrying device-side faults
            if attempt == 2:
                raise
            import gc
            import time as _time
            gc.collect()
            try:
                import jax
                jax.clear_caches()
                import jax.extend as _jex
                _jex.backend.clear_backends()
            except Exception:
                pass
            _time.sleep(10)


def prepare(x, weight, post_bin_scale, final_scale, final_bias, running_max,
            sprinkle_mask):
    """Build (compile) the module and the per-core input maps."""
    x = np.asarray(x, dtype=np.float32)
    weight = np.ascontiguousarray(np.asarray(weight, dtype=np.float32))
    mask = np.asarray(sprinkle_mask)
    pbs = np.asarray(post_bin_scale, dtype=np.float32)
    fs = np.asarray(final_scale, dtype=np.float32)
    fb = np.asarray(final_bias, dtype=np.float32)
    rm = float(np.asarray(running_max))

    if np.all(pbs.reshape(-1) == pbs.reshape(-1)[0]):
        inv_se, c0s, c1s, fscol, fbcol = _fast_consts(pbs, fs, fb, rm)
        nc = _build_fast(inv_se, c0s, c1s)
        maps = _fast_in_maps(x, weight, mask, fscol, fbcol)
        fast = True
    else:
        inv_se, c0, c1, fbb = _general_consts(pbs, fs, fb, rm)
        nc = _build_general(inv_se)
        maps = _general_in_maps(
            x, weight, np.ascontiguousarray(mask).view(np.uint8), c0, c1, fbb)
        fast = False
    return nc, maps, fast


def kernel(x, weight, post_bin_scale, final_scale, final_bias, running_max,
           sprinkle_mask):
    nc, maps, fast = prepare(x, weight, post_bin_scale, final_scale,
                             final_bias, running_max, sprinkle_mask)
    res = _run_spmd(nc, maps)
    if fast:
        out = np.concatenate(
            [np.asarray(res.results[c]["out"]).astype(np.float32).T
             for c in range(N_CORES)], axis=0)
    else:
        out = np.concatenate(
            [res.results[c]["out"] for c in range(N_CORES)], axis=0)
    return np.ascontiguousarray(out)


# revision 32
# speedup vs baseline: 1.4701x; 1.0005x over previous
"""BitLinear (quantized-activation, binarized-sprinkled-weight linear) Trainium2 kernel.

Data-parallel over the token dim N across 8 NeuronCores.

Fast path (post_bin_scale uniform, which holds for this problem's inputs):
  * all operands are laid out HOST-side (pure transposes/views) so the
    contraction dim i lands on SBUF partitions with no device transposes:
       xT [IN, N_SHARD] f32 (per core),  wT [IN, OUT] f32,  mT [IN, OUT] u8
  * w/x are DMA-cast to bf16 on load (gpsimd SWDGE, x in batched i-block
    pairs to amortize desc-gen; block 0 rides minimum-latency f32 paths);
    one fused DVE op per operand block:
       xb = t + clip(round(t), +-127),  t = x*inv_se          (bf16)
       W2 = m ? sign(w)*C0 : w*C1 + sign(w)*C0*0.5            (bf16)
    with C0 = sigma*pbs, C1 = 0.5*sigma global scalars (sigma = 0.5*(s+eps)),
    so that  out[n,o] = (xb @ W2^T)*fs[o] + fb[o]
  * prep is emitted in dependency order at half-block granularity so DVE
    production (the prep bottleneck, custom ops run 1x) matches the PE's
    PSUM-capacity-limited consumption exactly -- no matmul pacing stalls
  * matmuls produce PSUM tiles [o=128, n=512] (o on partitions), 16 k-steps.
    PSUM only holds 8 such tiles, so the contraction is split 8+8: during
    prep, every output tile's first half accumulates in waves of 8 and is
    parked in SBUF as bf16 partials; after prep the halves are reloaded
    (Act engine, software-pipelined two waves deep) and finished
  * epilogue on the Activation engine: Identity with per-partition scale=fs,
    bias=fb, writing bf16; stores go out transposed [OUT, N_SHARD] and the
    host transposes back / upcasts (layout-only work).

General path (non-uniform post_bin_scale): previous per-partition-constant
implementation, kept verbatim as a fallback.
"""

import numpy as np

N_CORES = 8
N_TOK, D_IN, D_OUT = 8192, 2048, 2048
N_SHARD = N_TOK // N_CORES          # 1024 rows of x per core
P = 128
NJ = N_SHARD // P                   # 8 n-blocks per core
NB_I = D_IN // P                    # 16 i-blocks (contraction)
NB_O = D_OUT // P                   # 16 o-blocks
OT = 512                            # o-tile (one PSUM bank)
NT = D_OUT // OT                    # 4 o-tiles
OB_PER_T = OT // P                  # 4 o-blocks per o-tile

MM_NT = N_SHARD // 512              # fast path: 2 moving-dim tiles of 512

QMAX = 127.0
EPS = 1e-6
MAGIC = 12582912.0                  # 1.5 * 2**23: fp32 RNE round-to-int trick

_CACHE = {}


def _register_ops():
    """Define the two fused DVE ops (idempotent)."""
    from concourse import dve_ops
    from concourse.dve_spec import (
        Spec, Src0, Src1, C0, C1, C2, Zero, select, minn, maxx, lower, _has_src1,
    )
    from concourse.dve_uop import DveOpSpec

    def register(name, spec):
        for op in dve_ops.OPS:
            if op.name == name:
                return op
        ver = "v3"
        tmp = DveOpSpec(name=name, opcode=0, uops=lower(spec, ver=ver),
                        rd1_en=_has_src1(spec))
        op = dve_ops.DveOp(name, spec, subdim=False,
                           uops_sha={ver: tmp.sha(ver)})
        dve_ops.OPS.append(op)
        dve_ops._SUB_OPCODE_FOR_NAME[name] = (
            max(dve_ops._SUB_OPCODE_FOR_NAME.values()) + 1)
        dve_ops.CUSTOM_DVE_SPECS[name] = spec
        return op

    # out = t + clip(round(t), +-imm2), t = x*s0   (s1 = MAGIC)
    _t = Src0 * C0
    _r = (_t + C1) - C1
    _rc = minn(maxx(_r, Zero - C2), C2)
    xprep = register("XPREP_BITLIN", Spec(
        body=_t + _rc,
        reference=lambda in0, in1, s0, s1, imm2: (
            (lambda t: t + np.clip(np.round(t), -imm2, imm2))(
                in0.astype(np.float32) * s0)),
    ))

    # h = select(w>=0, s0, -s0); out = select(m>0, h, w*s1 + h*imm2)
    _h = select(Src0 >= Zero, C0, Zero - C0)
    wprep = register("WPREP_BITLIN", Spec(
        body=select(Src1 > Zero, _h, Src0 * C1 + _h * C2),
        reference=lambda in0, in1, s0, s1, imm2: (
            (lambda h: np.where(in1 > 0, h,
                                in0.astype(np.float32) * s1 + h * imm2))(
                np.where(in0 >= 0, s0, -s0))),
    ))

    # two-constant variant (no imm2 -> allows 2-free-dim src1):
    # hh = select(w>=0, s0, -s0); out = select(m>0, hh+hh, w*s1 + hh)
    # with s0 = 0.5*C0 so that hh+hh = sign(w)*C0.
    _hh = select(Src0 >= Zero, C0, Zero - C0)
    wprep2 = register("WPREP2_BITLIN", Spec(
        body=select(Src1 > Zero, _hh + _hh, Src0 * C1 + _hh),
        reference=lambda in0, in1, s0, s1, imm2: (
            (lambda h: np.where(in1 > 0, h + h,
                                in0.astype(np.float32) * s1 + h))(
                np.where(in0 >= 0, s0, -s0))),
    ))
    return xprep, wprep, wprep2


def _build_fast(inv_se, c0s, c1s):
    """Fast-path per-core Bass module (uniform post_bin_scale).

    inv_se, c0s, c1s are global scalars baked into the DVE ops.
    """
    key = ("fast", float(inv_se), float(c0s), float(c1s))
    if key in _CACHE:
        return _CACHE[key]

    import concourse.mybir as mybir
    import concourse.tile as tile
    from concourse import bacc

    xprep, _, wprep2 = _register_ops()

    nc = bacc.Bacc(None, target_bir_lowering=False)
    bf16 = mybir.dt.bfloat16
    f32 = mybir.dt.float32
    ident = mybir.ActivationFunctionType.Identity

    x_in = nc.dram_tensor("x", [NB_I, P, N_SHARD], f32, kind="ExternalInput")
    w_in = nc.dram_tensor("w", [NB_I, P, D_OUT], f32, kind="ExternalInput")
    m_in = nc.dram_tensor("m", [NB_I, P, D_OUT], mybir.dt.uint8,
                          kind="ExternalInput")
    fs_in = nc.dram_tensor("fs", [P, NB_O], f32, kind="ExternalInput")
    fb_in = nc.dram_tensor("fb", [P, NB_O], f32, kind="ExternalInput")
    out_o = nc.dram_tensor("out", [D_OUT, N_SHARD], bf16, kind="ExternalOutput")

    KH = NB_I // 2                  # contraction split: 8 + 8 k-steps
    # wave order matches operand production order: (nt0,ob0-7) needs only the
    # Xa/Wa halves, (nt1,ob0-7) adds Xb, (nt0,ob8-15) adds Wb, (nt1,ob8-15)
    # needs everything -- so no wave ever waits on prep at a phase boundary
    tiles = ([(ob, 0) for ob in range(8)] + [(ob, 1) for ob in range(8)]
             + [(ob, 0) for ob in range(8, 16)]
             + [(ob, 1) for ob in range(8, 16)])
    waves = [tiles[w0:w0 + 8] for w0 in range(0, len(tiles), 8)]

    with tile.TileContext(nc) as tc:
        with (
            tc.tile_pool(name="persist", bufs=1) as persist,
            tc.tile_pool(name="wlp", bufs=6) as wlp,
            tc.tile_pool(name="mlp", bufs=6) as mlp,
            tc.tile_pool(name="xlp", bufs=5) as xlp,
            tc.tile_pool(name="ostage", bufs=8) as ostage,
            tc.tile_pool(name="psum", bufs=8, space="PSUM") as psum,
        ):
            w2 = persist.tile([P, NB_I, D_OUT], bf16, tag="w2")   # [i_in, ib, o]
            xb = persist.tile([P, NB_I, N_SHARD], bf16, tag="xb")  # [i_in, ib, n]
            # bf16 partial sums for the first contraction half (PSUM is only
            # 8 banks; parking partials in SBUF lets all 32 output tiles
            # overlap the operand-prep phase)
            parts = persist.tile([P, len(tiles), 512], bf16, tag="parts")
            fs_sb = persist.tile([P, NB_O], f32, tag="fs")
            fb_sb = persist.tile([P, NB_O], f32, tag="fb")

            # PE warm-up source: a zeroed bf16 tile for dummy matmuls that
            # keep the tensor engine busy through the prep-latency window so
            # real matmuls start at full p-state.
            warm = persist.tile([P, 512], bf16, tag="warm")
            nc.vector.memset(warm[:], 0.0)

            # Operand prep in dependency order. Phase-1 wave 1 (nt=0, ob 0-7)
            # consumes one (Xa, Wa) half-block pair per 1.7 us; emitting
            # exactly those halves first makes DVE production match PE
            # consumption, eliminating wave-1 pacing stalls. Remaining halves
            # follow in the order later waves need them.
            XHF, WHF = N_SHARD // 2, D_OUT // 2

            def x_op(ib, lo, sz, hwdge=False):
                # hwdge=True: plain f32 load on the sync HWDGE ring. Used in
                # the pacing-critical first phase so the Pool engine's SWDGE
                # desc-gen (994ns fixed per DMA) only carries the w loads and
                # operand production keeps up with wave-1 consumption.
                if hwdge:
                    xt = xlp.tile([P, sz], f32, name="xt", tag="x_f32")
                    nc.sync.dma_start(xt[:],
                                      x_in[ib, :, lo:lo + sz])
                else:
                    xt = xlp.tile([P, sz], bf16, name="xt", tag="x_bf16")
                    nc.gpsimd.dma_start(xt[:],
                                        x_in[ib, :, lo:lo + sz])
                nc.vector._custom_dve(
                    xprep, out=xb[:, ib, lo:lo + sz], in0=xt[:],
                    s0=float(inv_se), s1=MAGIC, imm2=QMAX)

            def w_op(ib, lo, sz):
                mt = mlp.tile([P, sz], mybir.dt.uint8, name="mt", tag="m_u8")
                nc.sync.dma_start(mt[:], m_in[ib, :, lo:lo + sz])
                wt = wlp.tile([P, sz], bf16, name="wt", tag="w_bf16")
                nc.gpsimd.dma_start(wt[:], w_in[ib, :, lo:lo + sz])
                nc.vector._custom_dve(
                    wprep2, out=w2[:, ib, lo:lo + sz], in0=wt[:], in1=mt[:],
                    s0=float(0.5 * c0s), s1=float(c1s))

            # Wave 1 deps, paced 1:1. Block 0 takes minimum-latency load
            # paths: its x half rides the sync HWDGE ring as plain f32 (no
            # Pool desc-gen), making the first w load the Pool generator's
            # FIRST job -- the first matmul starts ~1us sooner. Blocks 1-7's
            # x halves load two per SWDGE DMA (one desc-gen each),
            # prefetched ahead of use.
            xt0 = xlp.tile([P, XHF], f32, name="xt0", tag="x0_f32")
            nc.sync.dma_start(xt0[:], x_in[0, :, 0:XHF])
            wt0 = wlp.tile([P, WHF], bf16, name="wt0", tag="w_bf16")
            nc.gpsimd.dma_start(wt0[:], w_in[0, :, 0:WHF])
            mt0 = mlp.tile([P, WHF], mybir.dt.uint8, name="mt0", tag="m_u8")
            nc.sync.dma_start(mt0[:, 0:WHF // 2], m_in[0, :, 0:WHF // 2])
            nc.sync.dma_start(mt0[:, WHF // 2:], m_in[0, :, WHF // 2:WHF])

            xa_src = {}

            def xa_load(ib0, nb):
                t2 = xlp.tile([P, nb, XHF], bf16, name="t2", tag="x_bf16p")
                nc.gpsimd.dma_start(
                    t2[:], x_in[ib0:ib0 + nb, :, 0:XHF].transpose([1, 0, 2]))
                for j in range(nb):
                    xa_src[ib0 + j] = t2[:, j, :]

            xa_load(1, 2)
            nc.vector._custom_dve(
                xprep, out=xb[:, 0, 0:XHF], in0=xt0[:],
                s0=float(inv_se), s1=MAGIC, imm2=QMAX)
            # block-0 W half in two quarters: the first matmuls (ob 0-3)
            # need only w2[:, 0, 0:512], so they start ~0.5us sooner
            WQ = WHF // 2
            for q in range(2):
                nc.vector._custom_dve(
                    wprep2, out=w2[:, 0, q * WQ:(q + 1) * WQ],
                    in0=wt0[:, q * WQ:(q + 1) * WQ],
                    in1=mt0[:, q * WQ:(q + 1) * WQ],
                    s0=float(0.5 * c0s), s1=float(c1s))
            batches = [(3, 2), (5, 2), (7, 1)]
            bi = 0
            for ib in range(1, KH):
                nc.vector._custom_dve(
                    xprep, out=xb[:, ib, 0:XHF], in0=xa_src[ib],
                    s0=float(inv_se), s1=MAGIC, imm2=QMAX)
                w_op(ib, 0, WHF)
                if ib % 2 == 0 and bi < len(batches):
                    xa_load(*batches[bi])
                    bi += 1
            for ib in range(KH):                 # wave 2 deps (nt1, ob 0-7)
                x_op(ib, XHF, XHF)
            for ib in range(KH):                 # wave 3 deps (nt0, ob 8-15)
                w_op(ib, WHF, WHF)
            for ib in range(KH, NB_I):           # second contraction half
                x_op(ib, 0, N_SHARD)
                w_op(ib, 0, D_OUT)

            # epilogue constants (needed only after the first full tiles
            # finish, so loaded after the prep stream is underway)
            nc.scalar.dma_start(fs_sb[:], fs_in[:])
            nc.scalar.dma_start(fb_sb[:], fb_in[:])

            def mm(ps, ob, nt, ib, start, stop):
                nc.tensor.matmul(
                    ps[:],
                    w2[:, ib, ob * P:(ob + 1) * P],
                    xb[:, ib, nt * 512:(nt + 1) * 512],
                    start=start, stop=stop, skip_group_check=True)

            # phase 1 (overlaps prep): first contraction half for every tile,
            # k-outer within each 8-tile wave; partials parked in SBUF bf16.
            # The last wave's final 4 tiles keep their banks: phase 2
            # continues them in place while the first reloads pipeline in.
            held = []
            for wi, wave in enumerate(waves):
                pss = [psum.tile([P, 512], f32, name="ps", tag="ps")
                       for _ in wave]
                if wi == 0:
                    # dummies into pss[0]; the real ib-0 matmul resets it
                    # (start=True), so only timing is affected.
                    for _ in range(7):
                        nc.tensor.matmul(pss[0][:], warm[:, 0:P], warm[:],
                                         start=True, stop=True,
                                         skip_group_check=True)
                for ib in range(KH):
                    for ps, (ob, nt) in zip(pss, wave):
                        mm(ps, ob, nt, ib, ib == 0, ib == KH - 1)
                if wi == len(waves) - 1:
                    for k in range(4):
                        nc.scalar.copy(parts[:, wi * 8 + k, :], pss[k][:])
                    held = pss[4:]
                else:
                    for k, ps in enumerate(pss):
                        nc.scalar.copy(parts[:, wi * 8 + k, :], ps[:])

            def epilogue(ps, ob, nt):
                osb = ostage.tile([P, 512], bf16, tag="osb")
                nc.scalar.activation(
                    osb[:], ps[:], ident,
                    bias=fb_sb[:, ob:ob + 1], scale=fs_sb[:, ob:ob + 1])
                nc.sync.dma_start(
                    out_o[ob * P:(ob + 1) * P, nt * 512:(nt + 1) * 512],
                    osb[:])

            # phase 2: finish the 4 held tiles first (banks already hot) while
            # the first reload wave pipelines in on the Act queue; then parked
            # tiles in small waves, reloads running ahead of epilogues.
            parked = tiles[:28]
            held_tiles = tiles[28:]
            sizes = [4, 4, 4, 4, 4, 4, 2, 1, 1]
            p2, pos = [], 0
            for sz in sizes:
                p2.append(list(enumerate(parked[pos:pos + sz], start=pos)))
                pos += sz

            live_half = []

            def load_wave(wi):
                if wi == len(p2) - 1 and len(p2[wi]) == 1:
                    idx = p2[wi][0][0]
                    psA = psum.tile([P, 256], f32, name="psA", tag="ps")
                    psB = psum.tile([P, 256], f32, name="psB", tag="ps")
                    nc.scalar.copy(psA[:], parts[:, idx, 0:256])
                    nc.scalar.copy(psB[:], parts[:, idx, 256:512])
                    live_half.append(psB)
                    return [psA]
                pss = [psum.tile([P, 512], f32, name="ps", tag="ps")
                       for _ in p2[wi]]
                for ps, (idx, _) in zip(pss, p2[wi]):
                    nc.scalar.copy(ps[:], parts[:, idx, :])
                return pss

            live = {0: load_wave(0)}
            for ib in range(KH, NB_I):
                for ps, (ob, nt) in zip(held, held_tiles):
                    mm(ps, ob, nt, ib, False, ib == NB_I - 1)
            for ps, (ob, nt) in zip(held, held_tiles):
                epilogue(ps, ob, nt)
            live[1] = load_wave(1)

            for wi, wave in enumerate(p2):
                pss = live.pop(wi)
                if wi == len(p2) - 1 and len(wave) == 1:
                    # final tile in two independent 256-col halves: half A's
                    # epilogue+store launch while half B's matmuls run, and
                    # the closing drain chain shrinks to 256-wide ops
                    idx, (ob, nt) = wave[0]
                    psA, psB = pss[0], live_half[0]
                    for half, ps in ((0, psA), (1, psB)):
                        lo = nt * 512 + half * 256
                        for ib in range(KH, NB_I):
                            nc.tensor.matmul(
                                ps[:], w2[:, ib, ob * P:(ob + 1) * P],
                                xb[:, ib, lo:lo + 256],
                                start=False, stop=(ib == NB_I - 1),
                                skip_group_check=True)
                        osb = ostage.tile([P, 256], bf16, name="osbh",
                                          tag="osbh")
                        if half == 0:
                            nc.scalar.activation(
                                osb[:], ps[:], ident,
                                bias=fb_sb[:, ob:ob + 1],
                                scale=fs_sb[:, ob:ob + 1])
                        else:
                            # second half's scale+bias on the idle DVE so the
                            # two terminal epilogues run on different engines
                            nc.vector.tensor_scalar(
                                osb[:], ps[:], fs_sb[:, ob:ob + 1],
                                fb_sb[:, ob:ob + 1], mybir.AluOpType.mult,
                                mybir.AluOpType.add)
                        nc.sync.dma_start(
                            out_o[ob * P:(ob + 1) * P, lo:lo + 256], osb[:])
                    continue
                for ib in range(KH, NB_I):
                    for ps, (_, (ob, nt)) in zip(pss, wave):
                        mm(ps, ob, nt, ib, False, ib == NB_I - 1)
                for ps, (_, (ob, nt)) in zip(pss, wave):
                    epilogue(ps, ob, nt)
                if wi + 2 < len(p2):
                    live[wi + 2] = load_wave(wi + 2)

    nc.compile()
    _CACHE[key] = nc
    return nc


def _fast_consts(post_bin_scale, final_scale, final_bias, running_max):
    s = np.float32(running_max) / np.float32(QMAX)
    inv_se = np.float32(1.0) / (s + np.float32(EPS))
    sigma = np.float64(0.5) * (np.float64(s) + np.float64(EPS))
    pbs0 = np.float64(post_bin_scale.reshape(-1)[0])
    c0s = np.float32(sigma * pbs0)
    c1s = np.float32(np.float64(0.5) * sigma)
    fscol = np.ascontiguousarray(
        final_scale.astype(np.float32).reshape(NB_O, P).T)
    fbcol = np.ascontiguousarray(
        final_bias.astype(np.float32).reshape(NB_O, P).T)
    return inv_se, c0s, c1s, fscol, fbcol


def _fast_in_maps(x, weight, mask, fscol, fbcol):
    wT = np.ascontiguousarray(weight.T).reshape(NB_I, P, D_OUT)
    mT = np.ascontiguousarray(mask.T).view(np.uint8).reshape(NB_I, P, D_OUT)
    maps = []
    for c in range(N_CORES):
        xT = np.ascontiguousarray(x[c * N_SHARD:(c + 1) * N_SHARD].T)
        maps.append({
            "x": xT.reshape(NB_I, P, N_SHARD),
            "w": wT,
            "m": mT,
            "fs": fscol,
            "fb": fbcol,
        })
    return maps


# ---------------------------------------------------------------------------
# general fallback (non-uniform post_bin_scale): previous implementation
# ---------------------------------------------------------------------------

def _build_general(inv_se):
    key = ("nc", float(inv_se))
    if key in _CACHE:
        return _CACHE[key]

    import concourse.mybir as mybir
    import concourse.tile as tile
    from concourse import bacc

    xprep, wprep, _ = _register_ops()

    nc = bacc.Bacc(None, target_bir_lowering=False)
    bf16 = mybir.dt.bfloat16
    f32 = mybir.dt.float32

    x_in = nc.dram_tensor("x", [N_SHARD, D_IN], f32, kind="ExternalInput")
    w_in = nc.dram_tensor("w", [D_OUT, D_IN], f32, kind="ExternalInput")
    m_in = nc.dram_tensor("m", [D_OUT, D_IN], mybir.dt.uint8, kind="ExternalInput")
    c0_in = nc.dram_tensor("c0", [P, NB_O], f32, kind="ExternalInput")
    c1_in = nc.dram_tensor("c1", [P, NB_O], f32, kind="ExternalInput")
    fb_in = nc.dram_tensor("fb", [P, D_OUT], f32, kind="ExternalInput")
    out_o = nc.dram_tensor("out", [N_SHARD, D_OUT], f32, kind="ExternalOutput")

    from concourse.masks import make_identity

    with tile.TileContext(nc) as tc:
        with (
            tc.tile_pool(name="persist", bufs=1) as persist,
            tc.tile_pool(name="wlp", bufs=4) as wlp,
            tc.tile_pool(name="wpp", bufs=4) as wpp,
            tc.tile_pool(name="xlp", bufs=4) as xlp,
            tc.tile_pool(name="xbp", bufs=4) as xbp,
            tc.tile_pool(name="ostage", bufs=7) as ostage,
            tc.tile_pool(name="psum", bufs=6, space="PSUM") as psum,
            tc.tile_pool(name="tpsum", bufs=2, space="PSUM") as tpsum,
        ):
            wT = persist.tile([P, NB_O, NB_I, P], bf16, tag="wT")
            xqT = persist.tile([P, NJ, NB_I, P], bf16, tag="xqT")
            c0_sb = persist.tile([P, NB_O], f32, tag="c0")
            c1_sb = persist.tile([P, NB_O], f32, tag="c1")
            fb_sb = persist.tile([P, D_OUT], f32, tag="fb")
            ident = persist.tile([P, P], bf16, tag="ident")

            nc.sync.dma_start(fb_sb[:], fb_in[:])
            nc.sync.dma_start(c0_sb[:], c0_in[:])
            nc.sync.dma_start(c1_sb[:], c1_in[:])
            make_identity(nc, ident[:])

            def w_block(ob):
                wt = wlp.tile([P, D_IN], bf16, tag="w_bf16")
                mt = wlp.tile([P, D_IN], mybir.dt.uint8, tag="m_u8")
                nc.gpsimd.dma_start(wt[:], w_in[ob * P:(ob + 1) * P, :])
                nc.scalar.dma_start(mt[:], m_in[ob * P:(ob + 1) * P, :])
                w2 = wpp.tile([P, D_IN], bf16, tag="w2")
                nc.vector._custom_dve(
                    wprep, out=w2[:], in0=wt[:], in1=mt[:],
                    s0=c0_sb[:, ob:ob + 1], s1=c1_sb[:, ob:ob + 1], imm2=0.5)
                nc.sync.dma_start_transpose(wT[:, ob], w2[:])

            def x_block(j):
                xt = xlp.tile([P, D_IN], f32, tag="x_f32")
                nc.sync.dma_start(xt[:], x_in[j * P:(j + 1) * P, :])
                xb = xbp.tile([P, D_IN], bf16, tag="xb")
                nc.vector._custom_dve(
                    xprep, out=xb[:], in0=xt[:],
                    s0=float(inv_se), s1=MAGIC, imm2=QMAX)
                for b in range(NB_I):
                    tp = tpsum.tile([P, P], bf16, tag="xtp")
                    nc.tensor.transpose(tp[:], xb[:, b * P:(b + 1) * P], ident[:])
                    nc.scalar.copy(xqT[:, j, b, :], tp[:])

            for ob in range(OB_PER_T):
                w_block(ob)
            for j in range(NJ):
                x_block(j)
            for ob in range(OB_PER_T, NB_O):
                w_block(ob)

            for t in range(NT):
                for j in range(NJ):
                    ps = psum.tile([P, OT], f32, tag="ps")
                    for b in range(NB_I):
                        nc.tensor.matmul(
                            ps[:],
                            xqT[:, j, b, :],
                            wT[:, t * OB_PER_T:(t + 1) * OB_PER_T, b, :],
                            start=(b == 0), stop=(b == NB_I - 1))
                    osb = ostage.tile([P, OT], f32, tag="osb")
                    nc.vector.tensor_add(
                        osb[:], ps[:], fb_sb[:, t * OT:(t + 1) * OT])
                    nc.scalar.dma_start(
                        out_o[j * P:(j + 1) * P, t * OT:(t + 1) * OT], osb[:])

    nc.compile()
    _CACHE[key] = nc
    return nc


def _general_in_maps(x, weight, mask_u8, c0, c1, fb):
    maps = []
    for c in range(N_CORES):
        maps.append({
            "x": np.ascontiguousarray(x[c * N_SHARD:(c + 1) * N_SHARD]),
            "w": weight,
            "m": mask_u8,
            "c0": c0,
            "c1": c1,
            "fb": fb,
        })
    return maps


def _general_consts(post_bin_scale, final_scale, final_bias, running_max):
    s = np.float32(running_max) / np.float32(QMAX)
    inv_se = np.float32(1.0) / (s + np.float32(EPS))
    sigma = np.float64(0.5) * (np.float64(s) + np.float64(EPS))
    c0_all = (sigma * final_scale.astype(np.float64)
              * post_bin_scale.reshape(-1).astype(np.float64)).astype(np.float32)
    c1_all = (np.float64(0.5) * sigma
              * final_scale.astype(np.float64)).astype(np.float32)
    c0 = np.ascontiguousarray(c0_all.reshape(NB_O, P).T)
    c1 = np.ascontiguousarray(c1_all.reshape(NB_O, P).T)
    fb = np.ascontiguousarray(
        np.broadcast_to(final_bias.astype(np.float32), (P, D_OUT)))
    return inv_se, c0, c1, fb


def _run_spmd(nc, maps):
    """Execute with retry: axon-tunneled devices can transiently fail."""
    from concourse.bass_utils import run_bass_kernel_spmd
    for attempt in range(3):
        try:
            return run_bass_kernel_spmd(nc, maps, core_ids=list(range(N_CORES)))
        except Exception:  # noqa: BLE001 - retrying device-side faults
            if attempt == 2:
                raise
            import gc
            import time as _time
            gc.collect()
            try:
                import jax
                jax.clear_caches()
                import jax.extend as _jex
                _jex.backend.clear_backends()
            except Exception:
                pass
            _time.sleep(10)


def prepare(x, weight, post_bin_scale, final_scale, final_bias, running_max,
            sprinkle_mask):
    """Build (compile) the module and the per-core input maps."""
    x = np.asarray(x, dtype=np.float32)
    weight = np.ascontiguousarray(np.asarray(weight, dtype=np.float32))
    mask = np.asarray(sprinkle_mask)
    pbs = np.asarray(post_bin_scale, dtype=np.float32)
    fs = np.asarray(final_scale, dtype=np.float32)
    fb = np.asarray(final_bias, dtype=np.float32)
    rm = float(np.asarray(running_max))

    if np.all(pbs.reshape(-1) == pbs.reshape(-1)[0]):
        inv_se, c0s, c1s, fscol, fbcol = _fast_consts(pbs, fs, fb, rm)
        nc = _build_fast(inv_se, c0s, c1s)
        maps = _fast_in_maps(x, weight, mask, fscol, fbcol)
        fast = True
    else:
        inv_se, c0, c1, fbb = _general_consts(pbs, fs, fb, rm)
        nc = _build_general(inv_se)
        maps = _general_in_maps(
            x, weight, np.ascontiguousarray(mask).view(np.uint8), c0, c1, fbb)
        fast = False
    return nc, maps, fast


def kernel(x, weight, post_bin_scale, final_scale, final_bias, running_max,
           sprinkle_mask):
    nc, maps, fast = prepare(x, weight, post_bin_scale, final_scale,
                             final_bias, running_max, sprinkle_mask)
    res = _run_spmd(nc, maps)
    if fast:
        out = np.concatenate(
            [np.asarray(res.results[c]["out"]).astype(np.float32).T
             for c in range(N_CORES)], axis=0)
    else:
        out = np.concatenate(
            [res.results[c]["out"] for c in range(N_CORES)], axis=0)
    return np.ascontiguousarray(out)
